# revision 1
# baseline (speedup 1.0000x reference)
# Trainium2 Bass kernel for ByteCombineCNN — software-pipelined rewrite (v4).
#
# Same math as the baseline kernel (conv-as-dense-matmul + segmented maxpool,
# highway with ACT per-partition bias, projection bias via ones-row), emitted
# stage-major so no engine's in-order queue couples a group's late stages to
# the next group's early stages:
#   S0 load (SWDGE cast f32->bf16)       Pool ring       g = it
#   S1 input dma-xbar transpose          SP HWDGE        g = it-1
#   S2 conv matmuls (16)                 PE              g = it-2
#   S3 segmented maxpool (14 reduces)    DVE (+Pool)     g = it-3
#   S3b h transpose (PE) + bias-relu     PE + ACT        g = it-4
#   S4a highway layer 0                  PE + ACT/DVE    g = it-5
#   S4b highway layer 1                  PE + ACT/DVE/Pool  g = it-6
#   S5 projection + per-subtile store    PE + SP HWDGE   g = it-7
# The projection output is DMA'd directly from PSUM to HBM in f32 (no
# psum->sbuf copies, no output staging buffer, no host-side cast).
# PSUM: conv 2x2 banks + highway p/g 2 + {ht_ps,o_ps} shared 2 = 8 banks.
import numpy as np
import ml_dtypes

bf16 = ml_dtypes.bfloat16

B, T, BYTE_LEN, EMB = 8, 4096, 8, 64
FILTERS = [(1, 4), (2, 8), (3, 12), (4, 16), (5, 20), (6, 24), (7, 28)]
NPOS = [BYTE_LEN - w + 1 for w, _ in FILTERS]
# Filters merged into segments with a common (padded) position count so the
# segmented maxpool needs one reduce per segment instead of one per filter.
# Padded positions duplicate position 0 (max(a,a,...) == max(a,...)).
SEGS = [(8, [(1, 4), (2, 8)]), (6, [(3, 12), (4, 16)]),
        (4, [(5, 20), (6, 24)]), (2, [(7, 28)])]
LAST_DIM = 112
OUT_DIM = 512
FEAT = BYTE_LEN * EMB          # 512
CONV_COLS = sum(pp * sum(c for _, c in fl) for pp, fl in SEGS)  # 496
N_CORES = 8
S_PER_CORE = B * T // N_CORES  # 4096
GROUP = 512
NG = S_PER_CORE // GROUP       # 8
NST = GROUP // 128             # 4

_cache = {}


def _build(reps=1):
    import os
    import concourse.mybir as mybir
    import concourse.tile as tile
    from concourse import bacc
    from contextlib import ExitStack

    dt = mybir.dt
    nc = bacc.Bacc("TRN2", target_bir_lowering=False, debug=False)

    feat = nc.dram_tensor("features", [S_PER_CORE, FEAT], dt.float32, kind="ExternalInput").ap()
    wbig_d = nc.dram_tensor("wbig", [128, 4 * CONV_COLS], dt.bfloat16, kind="ExternalInput").ap()
    hwT_d = nc.dram_tensor("hwT", [112, 448], dt.bfloat16, kind="ExternalInput").ap()
    pwT_d = nc.dram_tensor("pwT", [128, 512], dt.bfloat16, kind="ExternalInput").ap()
    cbias_d = nc.dram_tensor("cbias", [112, 1], dt.float32, kind="ExternalInput").ap()
    hbias_d = nc.dram_tensor("hbias", [112, 4], dt.float32, kind="ExternalInput").ap()
    ident_d = nc.dram_tensor("ident", [128, 128], dt.bfloat16, kind="ExternalInput").ap()
    outp = nc.dram_tensor("out", [S_PER_CORE, OUT_DIM], dt.bfloat16, kind="ExternalOutput").ap()

    featv = feat.rearrange("(q p) f -> p q f", p=128)     # [128, 32 subtiles, 512]
    outv = outp.rearrange("(q p) o -> p q o", p=128)      # [128, 32 subtiles, 512]

    dev = os.environ.get("KDEV", "0") == "1"

    def _env(name, default):
        return os.environ.get(name, default) if dev else default

    def eng(name):
        return {"dve": nc.vector, "pool": nc.gpsimd, "act": nc.scalar}[name]

    ksub = _env("KSUB", "dve,dve").split(",")     # per-layer sub engine
    kmul = _env("KMUL", "dve,dve").split(",")      # per-layer mul engine
    kadd = _env("KADD", "dve,dve").split(",")     # per-layer add engine
    krelu = _env("KRELU", "act,act").split(",")    # per-layer relu engine
    kones = _env("KONES", "pool")                  # ones-rows memset engine
    kcopy = _env("KCOPY", "act,act,act,dve").split(",")  # per-st proj copy engine
    kpoolred = int(_env("KPOOLRED", "0"))          # first n filters' maxpool on Pool
    kord = _env("KORD", "S")                       # per-iteration stage emission order
    ktr = int(_env("KTR", "1"))                    # input xbar transposes per group
    kwarm = int(_env("KWARM", "0"))                # PE warm-up matmuls during fill
    kedge = _env("KEDGE", "0")                     # edge-group mode: 0/1/2
    ksplit = int(_env("KSPLIT", "2"))              # last n groups: split store
    kcv1 = _env("KCV1", "0") == "1"                # single 4-bank conv psum tile
    kdrelu = int(_env("KDRELU", "0"))              # last n groups: relu on DVE
    kpgsw = int(_env("KPGSW", "0"))                # swap p/g psum tags on layer 1
    kgs1 = _env("KGS1", "0") == "1"                # emit sigmoid before relu
    kmpi = _env("KMPI", "0") == "1"                # interleave maxpool reduces across pr
    kmpr = _env("KMPR", "0") == "1"                # reverse maxpool segment order
    kskew = _env("KSKEW", "compact")                  # pipeline skew table
    kbufs = {k: int(_env("KB_" + k, v)) for k, v in
             [("xg", "2"), ("xt", "5"), ("hraw", "3"), ("ht", "3"), ("act", "3")]}

    with tile.TileContext(nc) as tc, ExitStack() as ctx:
        const = ctx.enter_context(tc.tile_pool(name="const", bufs=1))
        wbig_sb = const.tile([128, 4, CONV_COLS], dt.bfloat16, name="wbig_sb")
        nc.sync.dma_start(out=wbig_sb[:], in_=wbig_d.rearrange("p (k c) -> p k c", k=4))
        hwT_sb = const.tile([112, 448], dt.bfloat16, name="hwT_sb")
        pwT_sb = const.tile([128, 512], dt.bfloat16, name="pwT_sb")
        cbias_sb = const.tile([112, 1], dt.float32, name="cbias_sb")
        hbias_sb = const.tile([112, 4], dt.float32, name="hbias_sb")
        ident_sb = const.tile([128, 128], dt.bfloat16, name="ident_sb")

        def load_late_consts():
            # emitted at iteration 2 and on the ACT HWDGE ring so the early
            # input transposes own the SP ring; first consumers run at
            # iteration 3+.
            nc.scalar.dma_start(out=hwT_sb[:], in_=hwT_d)
            nc.scalar.dma_start(out=pwT_sb[:], in_=pwT_d)
            nc.scalar.dma_start(out=cbias_sb[:], in_=cbias_d)
            nc.scalar.dma_start(out=hbias_sb[:], in_=hbias_d)
            nc.scalar.dma_start(out=ident_sb[:], in_=ident_d)

        xg_pool = ctx.enter_context(tc.tile_pool(name="xg", bufs=kbufs["xg"]))
        xt_pool = ctx.enter_context(tc.tile_pool(name="xt", bufs=kbufs["xt"]))
        conv_ps_pool = ctx.enter_context(tc.tile_pool(
            name="conv_ps", bufs=1 if kcv1 else 2, space="PSUM"))
        hraw_pool = ctx.enter_context(tc.tile_pool(name="hraw", bufs=kbufs["hraw"]))
        ht_pool = ctx.enter_context(tc.tile_pool(name="ht", bufs=kbufs["ht"]))
        act_pool = ctx.enter_context(tc.tile_pool(name="act", bufs=kbufs["act"]))
        pg_ps_pool = ctx.enter_context(tc.tile_pool(name="pg_ps", bufs=1, space="PSUM"))
        scr_ps_pool = ctx.enter_context(tc.tile_pool(name="scr_ps", bufs=2, space="PSUM"))
        out_pool = ctx.enter_context(tc.tile_pool(name="outsb", bufs=3))

        st_xg = {}
        st_xt = {}
        st_conv = {}
        st_hraw = {}
        st_ht = {}          # (g) -> hT after relu (input of layer 0)
        st_mid = {}         # (g) -> hT after layer 0
        st_fin = {}         # (g) -> hT_fin after layer 1

        def s0_load(g):
            q0, nst = sched[g]
            xg = xg_pool.tile([128, nst * FEAT], dt.bfloat16, name="xg", tag="xg")
            nc.gpsimd.dma_start(out=xg[:], in_=featv[:, q0:q0 + nst, :])
            st_xg[g] = xg

        def s1_xbar(g):
            q0, nst = sched[g]
            xt = xt_pool.tile([128, nst, 4, 128], dt.bfloat16, name="xt", tag="xt",
                              padded_shape=[128, NST, 4, 128])
            xg = st_xg.pop(g)
            nc.sync.dma_start_transpose(out=xt[:], in_=xg[:])
            st_xt[g] = xt

        def s2_conv(g):
            q0, nst = sched[g]
            xt = st_xt.pop(g)
            if kcv1:
                conv_ps = conv_ps_pool.tile([128, nst, 512], dt.float32,
                                            name="conv_ps", tag="conv_ps",
                                            padded_shape=[128, NST, 512])
                for stt in range(nst):
                    for kc in range(4):
                        nc.tensor.matmul(
                            conv_ps[:, stt, 0:CONV_COLS],
                            lhsT=xt[:, stt, kc, :],
                            rhs=wbig_sb[:, kc, :],
                            start=(kc == 0),
                            stop=(kc == 3),
                        )
                st_conv[g] = [conv_ps]
                return
            tiles = []
            for pr in range(nst // 2):
                conv_ps = conv_ps_pool.tile([128, 2, 512], dt.float32, name="conv_ps")
                for sub in range(2):
                    stt = pr * 2 + sub
                    for kc in range(4):
                        nc.tensor.matmul(
                            conv_ps[:, sub, 0:CONV_COLS],
                            lhsT=xt[:, stt, kc, :],
                            rhs=wbig_sb[:, kc, :],
                            start=(kc == 0),
                            stop=(kc == 3),
                        )
                tiles.append(conv_ps)
            st_conv[g] = tiles

        def s3_maxpool(g):
            q0, nst = sched[g]
            tiles = st_conv.pop(g)
            hraw = hraw_pool.tile([128, nst, LAST_DIM], dt.bfloat16, name="hraw",
                                  tag="hraw", padded_shape=[128, NST, LAST_DIM])
            bw = nst // len(tiles)  # subtiles covered per conv psum tile
            calls = []
            for pr, conv_ps in enumerate(tiles):
                off = 0
                offc = 0
                for si, (p_pad, flist) in enumerate(SEGS):
                    cseg = sum(c for _, c in flist)
                    calls.append((pr, si, conv_ps, off, offc, cseg, p_pad))
                    off += cseg * p_pad
                    offc += cseg
            if kmpi:
                calls.sort(key=lambda t: (t[1], t[0]))  # segment-major interleave
            if kmpr:
                calls.sort(key=lambda t: (t[0], -t[1]))
            for pr, si, conv_ps, off, offc, cseg, p_pad in calls:
                seg = conv_ps[:, 0:bw, off:off + cseg * p_pad].rearrange(
                    "a b (cc p) -> a b cc p", p=p_pad
                )
                nc.vector.tensor_reduce(
                    out=hraw[:, pr * bw:(pr + 1) * bw, offc:offc + cseg],
                    in_=seg,
                    axis=mybir.AxisListType.X,
                    op=mybir.AluOpType.max,
                )
            st_hraw[g] = hraw

        def s3b_htr(g):
            q0, nst = sched[g]
            ht_ps = scr_ps_pool.tile([112, NST, 128], dt.bfloat16, name="ht_ps", tag="scr")
            hraw = st_hraw.pop(g)
            for stt in range(nst):
                nc.tensor.transpose(ht_ps[:, stt, :], hraw[:, stt, :], ident_sb[:])
            hT = ht_pool.tile([112, nst * 128], dt.bfloat16, name="hT", tag="hT0",
                              padded_shape=[128, GROUP])
            nc.scalar.activation(
                hT[:], ht_ps[:, 0:nst].rearrange("a b c -> a (b c)"),
                mybir.ActivationFunctionType.Relu, bias=cbias_sb[:],
            )
            st_ht[g] = hT

        def s4_highway(g, l):
            q0, nst = sched[g]
            W = nst * 128
            hT = (st_ht if l == 0 else st_mid).pop(g)
            if l == 1:
                # allocate the output tile up-front so the ones-rows memset
                # runs off the critical l1 chain
                hT_out = ht_pool.tile([128, W], dt.bfloat16, name="hT_fin",
                                      tag="hT_fin", padded_shape=[128, GROUP])
                eng(kones).memset(hT_out[96:128, :], 1.0)
            tp, tg = ("p", "g") if (l == 0) == (kpgsw == 0) else ("g", "p")
            p_ps = pg_ps_pool.tile([112, GROUP], dt.float32, name="p_ps", tag=tp)
            g_ps = pg_ps_pool.tile([112, GROUP], dt.float32, name="g_ps", tag=tg)
            p_ps = p_ps[:, 0:W]
            g_ps = g_ps[:, 0:W]
            nc.tensor.matmul(p_ps, lhsT=hwT_sb[:, l * 224:l * 224 + 112],
                             rhs=hT[0:112, :], start=True, stop=True)
            nc.tensor.matmul(g_ps, lhsT=hwT_sb[:, l * 224 + 112:l * 224 + 224],
                             rhs=hT[0:112, :], start=True, stop=True)
            rp = act_pool.tile([112, W], dt.bfloat16, name="rp", tag=f"rp{l}",
                               padded_shape=[128, GROUP])
            gs = act_pool.tile([112, W], dt.bfloat16, name="gs", tag=f"gs{l}",
                               padded_shape=[128, GROUP])
            kr = krelu[l] if g < NGR - kdrelu else "dve"

            def emit_rp():
                if kr == "act":
                    nc.scalar.activation(rp[:], p_ps, mybir.ActivationFunctionType.Relu,
                                         bias=hbias_sb[:, 2 * l:2 * l + 1])
                else:
                    eng(kr).tensor_scalar(
                        out=rp[:], in0=p_ps, scalar1=hbias_sb[:, 2 * l:2 * l + 1],
                        scalar2=0.0, op0=mybir.AluOpType.add, op1=mybir.AluOpType.max)

            def emit_gs():
                nc.scalar.activation(gs[:], g_ps, mybir.ActivationFunctionType.Sigmoid,
                                     bias=hbias_sb[:, 2 * l + 1:2 * l + 2])

            if kgs1:
                emit_gs()
                emit_rp()
            else:
                emit_rp()
                emit_gs()
            d = act_pool.tile([112, W], dt.bfloat16, name="d", tag=f"d{l}",
                              padded_shape=[128, GROUP])
            eng(ksub[l]).tensor_sub(d[:], hT[0:112, :], rp[:])
            e = act_pool.tile([112, W], dt.bfloat16, name="e", tag=f"e{l}",
                              padded_shape=[128, GROUP])
            eng(kmul[l]).tensor_mul(e[:], gs[:], d[:])
            if l == 0:
                hT_next = ht_pool.tile([112, W], dt.bfloat16, name="hT_mid",
                                       tag="hT_mid", padded_shape=[128, GROUP])
                eng(kadd[l]).tensor_add(hT_next[0:112, :], e[:], rp[:])
                st_mid[g] = hT_next
            else:
                eng(kadd[l]).tensor_add(hT_out[0:112, :], e[:], rp[:])
                st_fin[g] = hT_out

        st_osb = {}

        def s5_proj(g):
            q0, nst = sched[g]
            hT = st_fin.pop(g)
            osb = out_pool.tile([128, nst, OUT_DIM], dt.bfloat16, name="osb",
                                tag="osb", padded_shape=[128, NST, OUT_DIM])
            # last group: alternate copy engines so the drain chain
            # (mm -> copy -> mm -> copy ...) overlaps instead of serializing
            # on ACT
            kc = ["act", "dve", "act", "dve"] if g >= NGR - ksplit else kcopy
            for stt in range(nst):
                o_ps = scr_ps_pool.tile([128, OUT_DIM], dt.float32, name="o_ps", tag="scr")
                nc.tensor.matmul(o_ps[:], lhsT=hT[:, stt * 128:(stt + 1) * 128],
                                 rhs=pwT_sb[:], start=True, stop=True)
                ce = kc[stt]
                if ce == "act":
                    nc.scalar.copy(out=osb[:, stt, :], in_=o_ps[:])
                elif ce == "dve":
                    nc.vector.tensor_copy(out=osb[:, stt, :], in_=o_ps[:])
                else:
                    nc.gpsimd.tensor_copy(out=osb[:, stt, :], in_=o_ps[:])
            st_osb[g] = osb

        def s6_store(g):
            q0, nst = sched[g]
            osb = st_osb.pop(g)
            if g >= NGR - ksplit and nst > 1:
                # split the final store so its first half overlaps the
                # second half's psum->sbuf copies
                h = nst // 2
                nc.sync.dma_start(out=outv[:, q0:q0 + h, :], in_=osb[:, 0:h])
                nc.sync.dma_start(out=outv[:, q0 + h:q0 + nst, :], in_=osb[:, h:nst])
            else:
                nc.sync.dma_start(out=outv[:, q0:q0 + nst, :], in_=osb[:])

        SKEWS = {
            "wide":    {"st": 8, "pj": 7, "h1": 6, "h0": 5, "tr": 4, "mp": 3, "cv": 2, "xb": 1, "ld": 0},
            "mid":     {"st": 7, "pj": 6, "h1": 5, "h0": 5, "tr": 4, "mp": 3, "cv": 2, "xb": 1, "ld": 0},
            "compact": {"st": 6, "pj": 5, "h1": 4, "h0": 4, "tr": 3, "mp": 3, "cv": 2, "xb": 1, "ld": 0},
            "c2":      {"st": 6, "pj": 5, "h1": 4, "h0": 4, "tr": 4, "mp": 3, "cv": 2, "xb": 1, "ld": 0},
            "tight":   {"st": 5, "pj": 4, "h1": 4, "h0": 4, "tr": 3, "mp": 3, "cv": 2, "xb": 1, "ld": 0},
        }[kskew]
        STAGES = {
            "st": (SKEWS["st"], s6_store), "pj": (SKEWS["pj"], s5_proj),
            "mp": (SKEWS["mp"], s3_maxpool),
            "h1": (SKEWS["h1"], lambda g: s4_highway(g, 1)),
            "h0": (SKEWS["h0"], lambda g: s4_highway(g, 0)),
            "tr": (SKEWS["tr"], s3b_htr), "cv": (SKEWS["cv"], s2_conv),
            "xb": (SKEWS["xb"], s1_xbar), "ld": (SKEWS["ld"], s0_load),
        }
        ORDERS = {
            "A": ["st", "pj", "mp", "h1", "h0", "tr", "cv", "xb", "ld"],
            "B": ["st", "pj", "mp", "tr", "cv", "h1", "h0", "xb", "ld"],
            "C": ["st", "pj", "mp", "cv", "h1", "h0", "tr", "xb", "ld"],
            "D": ["st", "pj", "h1", "h0", "mp", "tr", "cv", "xb", "ld"],
            "E": ["st", "pj", "h1", "mp", "cv", "h0", "tr", "xb", "ld"],
            "F": ["st", "pj", "mp", "h1", "cv", "h0", "tr", "xb", "ld"],
            "G": ["st", "h1", "pj", "mp", "cv", "h0", "tr", "xb", "ld"],
            "H": ["st", "h1", "pj", "mp", "h0", "cv", "tr", "xb", "ld"],
            "I": ["st", "mp", "pj", "h1", "cv", "h0", "tr", "xb", "ld"],
            "J": ["st", "mp", "h1", "pj", "cv", "h0", "tr", "xb", "ld"],
            "K": ["st", "mp", "pj", "h1", "h0", "cv", "tr", "xb", "ld"],
            "L": ["st", "mp", "h1", "cv", "pj", "h0", "tr", "xb", "ld"],
            "M": ["st", "mp", "h1", "pj", "cv", "tr", "h0", "xb", "ld"],
            "N": ["st", "mp", "h1", "pj", "h0", "cv", "tr", "xb", "ld"],
            "O": ["mp", "st", "h1", "pj", "cv", "h0", "tr", "xb", "ld"],
            "V": ["st", "mp", "pj", "h0", "h1", "cv", "tr", "xb", "ld"],
            "W": ["mp", "st", "pj", "h0", "h1", "tr", "cv", "xb", "ld"],
            "P": ["st", "mp", "pj", "cv", "h0", "h1", "tr", "xb", "ld"],
            "Q": ["st", "mp", "cv", "pj", "h0", "h1", "tr", "xb", "ld"],
            "R": ["st", "mp", "pj", "h0", "h1", "cv", "tr", "xb", "ld"],
            "S": ["st", "mp", "pj", "h0", "h1", "tr", "cv", "xb", "ld"],
            "T": ["st", "mp", "cv", "h0", "h1", "pj", "tr", "xb", "ld"],
            "U": ["st", "mp", "h0", "h1", "pj", "cv", "tr", "xb", "ld"],
        }
        if kwarm:
            # keep PE busy from t=0 so the HAM clock gate releases before the
            # first conv group arrives (pg psum bank is unused during fill)
            warm_ps = pg_ps_pool.tile([112, GROUP], dt.float32, name="p_ps", tag="p")
            for _ in range(kwarm):
                nc.tensor.matmul(warm_ps[:, 0:64], lhsT=hwT_sb[:, 0:112],
                                 rhs=hwT_sb[:, 0:64], start=True, stop=True)

        if kedge == "1":
            base = [(0, 2), (2, 2)] + [(4 + 4 * i, 4) for i in range(6)] + [(28, 2), (30, 2)]
        elif kedge == "2":
            # drain-only: halve just the final group
            base = [(4 * i, 4) for i in range(NG - 1)] + [(28, 2), (30, 2)]
        else:
            base = [(4 * i, 4) for i in range(NG)]
        sched = []
        for r in range(reps):
            sched.extend(base)
        NGR = len(sched)
        for it in range(NGR + 8):
            if it == 2:
                load_late_consts()
            for key in ORDERS[kord]:
                skew, fn = STAGES[key]
                g = it - skew
                if 0 <= g < NGR:
                    fn(g)

    nc.compile()
    return nc


def _prep_weights(inputs):
    W = np.zeros((FEAT, CONV_COLS), np.float32)
    cb = np.zeros(LAST_DIM, np.float32)
    off = 0
    offc = 0
    for p_pad, flist in SEGS:
        for w, c in flist:
            i = w  # filter index == width for this problem
            p_i = BYTE_LEN - w + 1
            cw = np.asarray(inputs[f"conv_w{i}"], np.float32)  # [c, EMB, w]
            for p in range(p_pad):
                sp = p if p < p_i else 0  # duplicate position 0 as padding
                for k in range(w):
                    byte = sp + k
                    W[byte * EMB:(byte + 1) * EMB,
                      off + p:off + c * p_pad:p_pad] = cw[:, :, k].T
            cb[offc:offc + c] = np.asarray(inputs[f"conv_b{i}"], np.float32)
            off += c * p_pad
            offc += c
    wbig = np.ascontiguousarray(
        W.reshape(4, 128, CONV_COLS).transpose(1, 0, 2).reshape(128, 4 * CONV_COLS)
    ).astype(bf16)
    hwT = np.concatenate([np.asarray(inputs["hw_w1"], np.float32).T,
                          np.asarray(inputs["hw_w2"], np.float32).T], 1)
    hwT = np.ascontiguousarray(hwT).astype(bf16)  # [112, 448]
    pwT = np.zeros((128, 512), np.float32)
    pwT[:112] = np.asarray(inputs["proj_w"], np.float32).T
    pwT[112] = np.asarray(inputs["proj_b"], np.float32)
    pwT = np.ascontiguousarray(pwT).astype(bf16)
    hb1 = np.asarray(inputs["hw_b1"], np.float32)
    hb2 = np.asarray(inputs["hw_b2"], np.float32)
    hbias = np.stack([hb1[:112], hb1[112:], hb2[:112], hb2[112:]], 1)  # [112, 4]
    hbias = np.ascontiguousarray(hbias)
    return wbig, hwT, pwT, cb.reshape(112, 1), hbias


def _in_maps(inputs):
    wbig, hwT, pwT, cb, hbias = _prep_weights(inputs)
    ident = np.eye(128, dtype=bf16)
    feats = np.ascontiguousarray(
        np.asarray(inputs["features"], np.float32).reshape(B * T, FEAT)
    )
    return [{
        "features": feats[c * S_PER_CORE:(c + 1) * S_PER_CORE],
        "wbig": wbig, "hwT": hwT, "pwT": pwT, "cbias": cb, "hbias": hbias,
        "ident": ident,
    } for c in range(N_CORES)]


def kernel(**inputs) -> np.ndarray:
    from concourse.bass_utils import run_bass_kernel_spmd

    if "nc" not in _cache:
        _cache["nc"] = _build()
    nc = _cache["nc"]

    in_maps = _in_maps(inputs)
    res = run_bass_kernel_spmd(nc, in_maps, core_ids=list(range(N_CORES)))
    out = np.concatenate([res.results[c]["out"] for c in range(N_CORES)], 0)
    return np.ascontiguousarray(out.reshape(B, T, OUT_DIM)).astype(np.float32)



# revision 24
# speedup vs baseline: 1.0850x; 1.0850x over previous
# Trainium2 Bass kernel for ByteCombineCNN — software-pipelined rewrite (v4).
#
# Same math as the baseline kernel (conv-as-dense-matmul + segmented maxpool,
# highway with ACT per-partition bias, projection bias via ones-row), emitted
# stage-major so no engine's in-order queue couples a group's late stages to
# the next group's early stages:
#   S0 load (SWDGE cast f32->bf16)       Pool ring       g = it
#   S1 input dma-xbar transpose          SP HWDGE        g = it-1
#   S2 conv matmuls (16)                 PE              g = it-2
#   S3 segmented maxpool (14 reduces)    DVE (+Pool)     g = it-3
#   S3b h transpose (PE) + bias-relu     PE + ACT        g = it-4
#   S4a highway layer 0                  PE + ACT/DVE    g = it-5
#   S4b highway layer 1                  PE + ACT/DVE/Pool  g = it-6
#   S5 projection + per-subtile store    PE + SP HWDGE   g = it-7
# The projection output is DMA'd directly from PSUM to HBM in f32 (no
# psum->sbuf copies, no output staging buffer, no host-side cast).
# PSUM: conv 2x2 banks + highway p/g 2 + {ht_ps,o_ps} shared 2 = 8 banks.
import numpy as np
import ml_dtypes

bf16 = ml_dtypes.bfloat16

B, T, BYTE_LEN, EMB = 8, 4096, 8, 64
FILTERS = [(1, 4), (2, 8), (3, 12), (4, 16), (5, 20), (6, 24), (7, 28)]
NPOS = [BYTE_LEN - w + 1 for w, _ in FILTERS]
# Filters merged into segments with a common (padded) position count so the
# segmented maxpool needs one reduce per segment instead of one per filter.
# Padded positions duplicate position 0 (max(a,a,...) == max(a,...)).
SEGS = [(8, [(1, 4), (2, 8)]), (6, [(3, 12), (4, 16)]),
        (4, [(5, 20), (6, 24)]), (2, [(7, 28)])]
LAST_DIM = 112
OUT_DIM = 512
FEAT = BYTE_LEN * EMB          # 512
CONV_COLS = sum(pp * sum(c for _, c in fl) for pp, fl in SEGS)  # 496
N_CORES = 8
S_PER_CORE = B * T // N_CORES  # 4096
import os as _os
GROUP = int(_os.environ.get("KGRP", "512")) if _os.environ.get("KDEV", "0") == "1" else 512
NG = S_PER_CORE // GROUP       # 8
NST = GROUP // 128             # 4

_cache = {}


def _build(reps=1):
    import os
    import concourse.mybir as mybir
    import concourse.tile as tile
    from concourse import bacc
    from contextlib import ExitStack

    dt = mybir.dt
    nc = bacc.Bacc("TRN2", target_bir_lowering=False, debug=False)

    feat = nc.dram_tensor("features", [S_PER_CORE, FEAT], dt.float32, kind="ExternalInput").ap()
    wbig_d = nc.dram_tensor("wbig", [128, 4 * CONV_COLS], dt.bfloat16, kind="ExternalInput").ap()
    hwT_d = nc.dram_tensor("hwT", [112, 448], dt.bfloat16, kind="ExternalInput").ap()
    pwT_d = nc.dram_tensor("pwT", [128, 512], dt.bfloat16, kind="ExternalInput").ap()
    cbias_d = nc.dram_tensor("cbias", [112, 1], dt.float32, kind="ExternalInput").ap()
    hbias_d = nc.dram_tensor("hbias", [112, 4], dt.float32, kind="ExternalInput").ap()
    ident_d = nc.dram_tensor("ident", [128, 128], dt.bfloat16, kind="ExternalInput").ap()
    outp = nc.dram_tensor("out", [S_PER_CORE, OUT_DIM], dt.bfloat16, kind="ExternalOutput").ap()

    featv = feat.rearrange("(q p) f -> p q f", p=128)     # [128, 32 subtiles, 512]
    outv = outp.rearrange("(q p) o -> p q o", p=128)      # [128, 32 subtiles, 512]

    dev = os.environ.get("KDEV", "0") == "1"

    def _env(name, default):
        return os.environ.get(name, default) if dev else default

    def eng(name):
        return {"dve": nc.vector, "pool": nc.gpsimd, "act": nc.scalar}[name]

    kabl = set(_env("KABL", "").split(",")) - {""}  # ablate stages (sim probe)
    kv5 = _env("KV5", "0") == "1"                  # upfront input stream driver
    kv5sk = _env("KV5SK", "cv:0,mp:1,tr:1,h0:2,h1:2,pj:3,st:3")
    kv5ord = _env("KV5ORD", "st,pj,h1,h0,tr,mp,cv")
    kstq = _env("KSTQ", "sp")                      # store queue: sp|pool|act|dve
    kpre = int(_env("KPRE", "1"))                  # preload ACT tables at t=0
    kfill = int(_env("KFILL", "0"))                # filler matmuls per fill iter
    kfillpre = int(_env("KFILLPRE", "70"))         # fillers emitted before loop
    kpair = _env("KPAIR", "0") == "1"              # pair groups in tr/hw stages
    ksub = _env("KSUB", "dve,dve").split(",")     # per-layer sub engine
    kmul = _env("KMUL", "dve,dve").split(",")      # per-layer mul engine
    kadd = _env("KADD", "dve,dve").split(",")     # per-layer add engine
    krelu = _env("KRELU", "act,act").split(",")    # per-layer relu engine
    kones = _env("KONES", "pool")                  # ones-rows memset engine
    kcopy = _env("KCOPY", "act,act,act,dve").split(",")  # per-st proj copy engine
    kpoolred = int(_env("KPOOLRED", "0"))          # first n filters' maxpool on Pool
    kord = _env("KORD", "S")                       # per-iteration stage emission order
    ktr = int(_env("KTR", "1"))                    # input xbar transposes per group
    kwarm = int(_env("KWARM", "0"))                # PE warm-up matmuls during fill
    kedge = _env("KEDGE", "0")                     # edge-group mode: 0/1/2
    ksplit = int(_env("KSPLIT", "2"))              # last n groups: split store
    kcv1 = _env("KCV1", "0") == "1"                # single 4-bank conv psum tile
    kdrelu = int(_env("KDRELU", "0"))              # last n groups: relu on DVE
    kpgsw = int(_env("KPGSW", "0"))                # swap p/g psum tags on layer 1
    kgs1 = _env("KGS1", "0") == "1"                # emit sigmoid before relu
    kmpi = _env("KMPI", "0") == "1"                # interleave maxpool reduces across pr
    kmpr = _env("KMPR", "0") == "1"                # reverse maxpool segment order
    kskew = _env("KSKEW", "compact")                  # pipeline skew table
    kbufs = {k: int(_env("KB_" + k, v)) for k, v in
             [("xg", "2"), ("xt", "5"), ("hraw", "3"), ("ht", "5"), ("act", "5")]}
    if kv5:
        # upfront input streaming needs every group's staging + transposed
        # tile resident at once
        kbufs["xg"] = NG
        kbufs["xt"] = NG

    with tile.TileContext(nc) as tc, ExitStack() as ctx:
        const = ctx.enter_context(tc.tile_pool(name="const", bufs=1))
        wbig_sb = const.tile([128, 4, CONV_COLS], dt.bfloat16, name="wbig_sb")
        nc.sync.dma_start(out=wbig_sb[:], in_=wbig_d.rearrange("p (k c) -> p k c", k=4))
        hwT_sb = const.tile([112, 448], dt.bfloat16, name="hwT_sb")
        pwT_sb = const.tile([128, 512], dt.bfloat16, name="pwT_sb")
        cbias_sb = const.tile([112, 1], dt.float32, name="cbias_sb")
        hbias_sb = const.tile([112, 4], dt.float32, name="hbias_sb")
        ident_sb = const.tile([128, 128], dt.bfloat16, name="ident_sb")

        pre_sb = const.tile([112, 1], dt.bfloat16, name="pre_sb") if kpre else None

        def load_early_consts():
            # tiny biases first so the ACT-table preload dummies have real
            # operands; the big weights stay at iteration 2.
            nc.scalar.dma_start(out=cbias_sb[:], in_=cbias_d)
            nc.scalar.dma_start(out=hbias_sb[:], in_=hbias_d)
            # trigger every ACT function-set load while the pipe is filling
            nc.scalar.activation(pre_sb[:], cbias_sb[:],
                                 mybir.ActivationFunctionType.Copy)
            nc.scalar.activation(pre_sb[:], cbias_sb[:],
                                 mybir.ActivationFunctionType.Relu,
                                 bias=cbias_sb[:])
            nc.scalar.activation(pre_sb[:], cbias_sb[:],
                                 mybir.ActivationFunctionType.Sigmoid,
                                 bias=cbias_sb[:])

        def load_late_consts():
            # emitted at iteration 2 and on the ACT HWDGE ring so the early
            # input transposes own the SP ring; first consumers run at
            # iteration 3+.
            nc.scalar.dma_start(out=hwT_sb[:], in_=hwT_d)
            nc.scalar.dma_start(out=pwT_sb[:], in_=pwT_d)
            if not kpre:
                nc.scalar.dma_start(out=cbias_sb[:], in_=cbias_d)
                nc.scalar.dma_start(out=hbias_sb[:], in_=hbias_d)
            nc.scalar.dma_start(out=ident_sb[:], in_=ident_d)

        xg_pool = ctx.enter_context(tc.tile_pool(name="xg", bufs=kbufs["xg"]))
        xt_pool = ctx.enter_context(tc.tile_pool(name="xt", bufs=kbufs["xt"]))
        conv_ps_pool = ctx.enter_context(tc.tile_pool(
            name="conv_ps", bufs=1 if kcv1 else 2, space="PSUM"))
        hraw_pool = ctx.enter_context(tc.tile_pool(name="hraw", bufs=kbufs["hraw"]))
        ht_pool = ctx.enter_context(tc.tile_pool(name="ht", bufs=kbufs["ht"]))
        act_pool = ctx.enter_context(tc.tile_pool(name="act", bufs=kbufs["act"]))
        pg_ps_pool = ctx.enter_context(tc.tile_pool(name="pg_ps", bufs=1, space="PSUM"))
        scr_ps_pool = ctx.enter_context(tc.tile_pool(name="scr_ps", bufs=2, space="PSUM"))
        out_pool = ctx.enter_context(tc.tile_pool(name="outsb", bufs=3))

        st_xg = {}
        st_xt = {}
        st_conv = {}
        st_hraw = {}
        st_ht = {}          # (g) -> hT after relu (input of layer 0)
        st_mid = {}         # (g) -> hT after layer 0
        st_fin = {}         # (g) -> hT_fin after layer 1

        def s0_load(g):
            q0, nst = sched[g]
            xg = xg_pool.tile([128, nst * FEAT], dt.bfloat16, name="xg", tag="xg")
            if "ld" in kabl:
                nc.gpsimd.dma_start(out=xg[:, 0:1], in_=featv[:, q0:q0 + 1, 0:1])
            else:
                nc.gpsimd.dma_start(out=xg[:], in_=featv[:, q0:q0 + nst, :])
            st_xg[g] = xg

        def s1_xbar(g):
            q0, nst = sched[g]
            xt = xt_pool.tile([128, nst, 4, 128], dt.bfloat16, name="xt", tag="xt",
                              padded_shape=[128, NST, 4, 128])
            xg = st_xg.pop(g)
            if "xb" in kabl:
                nc.sync.dma_start_transpose(
                    out=xt[0:64, 0, 0, :], in_=xg[:, 0:64])
            else:
                nc.sync.dma_start_transpose(out=xt[:], in_=xg[:])
            st_xt[g] = xt

        def s2_conv(g):
            q0, nst = sched[g]
            xt = st_xt.pop(g)
            if kcv1:
                conv_ps = conv_ps_pool.tile([128, nst, 512], dt.float32,
                                            name="conv_ps", tag="conv_ps",
                                            padded_shape=[128, NST, 512])
                for stt in range(nst):
                    for kc in range(4):
                        if "cv" in kabl: continue
                        nc.tensor.matmul(
                            conv_ps[:, stt, 0:CONV_COLS],
                            lhsT=xt[:, stt, kc, :],
                            rhs=wbig_sb[:, kc, :],
                            start=(kc == 0),
                            stop=(kc == 3),
                        )
                st_conv[g] = [conv_ps]
                return
            tiles = []
            pw = min(2, nst)  # subtiles per conv psum tile
            for pr in range(nst // pw):
                conv_ps = conv_ps_pool.tile([128, pw, 512], dt.float32, name="conv_ps")
                for sub in range(pw):
                    stt = pr * pw + sub
                    for kc in range(4):
                        if "cv" in kabl:
                            if kc == 0:
                                nc.tensor.matmul(
                                    conv_ps[:, sub, 0:1],
                                    lhsT=xt[:, stt, 0, :],
                                    rhs=wbig_sb[:, 0, 0:1],
                                    start=True, stop=True)
                            continue
                        nc.tensor.matmul(
                            conv_ps[:, sub, 0:CONV_COLS],
                            lhsT=xt[:, stt, kc, :],
                            rhs=wbig_sb[:, kc, :],
                            start=(kc == 0),
                            stop=(kc == 3),
                        )
                tiles.append(conv_ps)
            st_conv[g] = tiles

        def s3_maxpool(g):
            q0, nst = sched[g]
            tiles = st_conv.pop(g)
            hraw = hraw_pool.tile([128, nst, LAST_DIM], dt.bfloat16, name="hraw",
                                  tag="hraw", padded_shape=[128, NST, LAST_DIM])
            bw = nst // len(tiles)  # subtiles covered per conv psum tile
            calls = []
            for pr, conv_ps in enumerate(tiles):
                off = 0
                offc = 0
                for si, (p_pad, flist) in enumerate(SEGS):
                    cseg = sum(c for _, c in flist)
                    calls.append((pr, si, conv_ps, off, offc, cseg, p_pad))
                    off += cseg * p_pad
                    offc += cseg
            if kmpi:
                calls.sort(key=lambda t: (t[1], t[0]))  # segment-major interleave
            if kmpr:
                calls.sort(key=lambda t: (t[0], -t[1]))
            for pr, si, conv_ps, off, offc, cseg, p_pad in calls:
                if "mp" in kabl:
                    p_pad = 1
                seg = conv_ps[:, 0:bw, off:off + cseg * p_pad].rearrange(
                    "a b (cc p) -> a b cc p", p=p_pad
                )
                nc.vector.tensor_reduce(
                    out=hraw[:, pr * bw:(pr + 1) * bw, offc:offc + cseg],
                    in_=seg,
                    axis=mybir.AxisListType.X,
                    op=mybir.AluOpType.max,
                )
            st_hraw[g] = hraw

        def s3b_htr(g):
            q0, nst = sched[g]
            ht_ps = scr_ps_pool.tile([112, NST, 128], dt.bfloat16, name="ht_ps", tag="scr")
            hraw = st_hraw.pop(g)
            for stt in range(nst):
                if "trp" in kabl: continue
                nc.tensor.transpose(ht_ps[:, stt, :], hraw[:, stt, :], ident_sb[:])
            hT = ht_pool.tile([112, nst * 128], dt.bfloat16, name="hT", tag="hT0",
                              padded_shape=[128, GROUP])
            if "tra" in kabl:
                nc.scalar.activation(
                    hT[:, 0:1], ht_ps[:, 0, 0:1],
                    mybir.ActivationFunctionType.Relu, bias=cbias_sb[:],
                )
            else:
                nc.scalar.activation(
                    hT[:], ht_ps[:, 0:nst].rearrange("a b c -> a (b c)"),
                    mybir.ActivationFunctionType.Relu, bias=cbias_sb[:],
                )
            st_ht[g] = hT

        def s4_highway(g, l):
            q0, nst = sched[g]
            W = nst * 128
            hT = (st_ht if l == 0 else st_mid).pop(g)
            if l == 1:
                # allocate the output tile up-front so the ones-rows memset
                # runs off the critical l1 chain
                hT_out = ht_pool.tile([128, W], dt.bfloat16, name="hT_fin",
                                      tag="hT_fin", padded_shape=[128, GROUP])
                eng(kones).memset(hT_out[96:128, :], 1.0)
            tp, tg = ("p", "g") if (l == 0) == (kpgsw == 0) else ("g", "p")
            p_ps = pg_ps_pool.tile([112, GROUP], dt.float32, name="p_ps", tag=tp)
            g_ps = pg_ps_pool.tile([112, GROUP], dt.float32, name="g_ps", tag=tg)
            p_ps = p_ps[:, 0:W]
            g_ps = g_ps[:, 0:W]
            if "hwmm" in kabl:
                nc.tensor.matmul(p_ps[:, 0:1], lhsT=hwT_sb[:, l * 224:l * 224 + 112],
                                 rhs=hT[0:112, 0:1], start=True, stop=True)
                nc.tensor.matmul(g_ps[:, 0:1], lhsT=hwT_sb[:, l * 224 + 112:l * 224 + 224],
                                 rhs=hT[0:112, 0:1], start=True, stop=True)
            else:
                nc.tensor.matmul(p_ps, lhsT=hwT_sb[:, l * 224:l * 224 + 112],
                                 rhs=hT[0:112, :], start=True, stop=True)
                nc.tensor.matmul(g_ps, lhsT=hwT_sb[:, l * 224 + 112:l * 224 + 224],
                                 rhs=hT[0:112, :], start=True, stop=True)
            rp = act_pool.tile([112, W], dt.bfloat16, name="rp", tag=f"rp{l}",
                               padded_shape=[128, GROUP])
            gs = act_pool.tile([112, W], dt.bfloat16, name="gs", tag=f"gs{l}",
                               padded_shape=[128, GROUP])
            kr = krelu[l] if g < NGR - kdrelu else "dve"

            def emit_rp():
                if "hwrelu" in kabl:
                    nc.scalar.activation(rp[:, 0:1], p_ps[:, 0:1],
                                         mybir.ActivationFunctionType.Relu,
                                         bias=hbias_sb[:, 2 * l:2 * l + 1])
                    return
                if kr == "act":
                    nc.scalar.activation(rp[:], p_ps, mybir.ActivationFunctionType.Relu,
                                         bias=hbias_sb[:, 2 * l:2 * l + 1])
                else:
                    eng(kr).tensor_scalar(
                        out=rp[:], in0=p_ps, scalar1=hbias_sb[:, 2 * l:2 * l + 1],
                        scalar2=0.0, op0=mybir.AluOpType.add, op1=mybir.AluOpType.max)

            def emit_gs():
                if "hwsig" in kabl:
                    nc.scalar.activation(gs[:, 0:1], g_ps[:, 0:1],
                                         mybir.ActivationFunctionType.Sigmoid,
                                         bias=hbias_sb[:, 2 * l + 1:2 * l + 2])
                    return
                nc.scalar.activation(gs[:], g_ps, mybir.ActivationFunctionType.Sigmoid,
                                     bias=hbias_sb[:, 2 * l + 1:2 * l + 2])

            if kgs1:
                emit_gs()
                emit_rp()
            else:
                emit_rp()
                emit_gs()
            d = act_pool.tile([112, W], dt.bfloat16, name="d", tag=f"d{l}",
                              padded_shape=[128, GROUP])
            if "hwtt" in kabl:
                eng(ksub[l]).tensor_sub(d[:, 0:1], hT[0:112, 0:1], rp[:, 0:1])
            else:
                eng(ksub[l]).tensor_sub(d[:], hT[0:112, :], rp[:])
            e = act_pool.tile([112, W], dt.bfloat16, name="e", tag=f"e{l}",
                              padded_shape=[128, GROUP])
            if "hwtt" in kabl:
                eng(kmul[l]).tensor_mul(e[:, 0:1], gs[:, 0:1], d[:, 0:1])
            else:
                eng(kmul[l]).tensor_mul(e[:], gs[:], d[:])
            if l == 0:
                hT_next = ht_pool.tile([112, W], dt.bfloat16, name="hT_mid",
                                       tag="hT_mid", padded_shape=[128, GROUP])
                if "hwtt" in kabl:
                    eng(kadd[l]).tensor_add(hT_next[0:112, 0:1], e[:, 0:1], rp[:, 0:1])
                else:
                    eng(kadd[l]).tensor_add(hT_next[0:112, :], e[:], rp[:])
                st_mid[g] = hT_next
            else:
                if "hwtt" in kabl:
                    eng(kadd[l]).tensor_add(hT_out[0:112, 0:1], e[:, 0:1], rp[:, 0:1])
                else:
                    eng(kadd[l]).tensor_add(hT_out[0:112, :], e[:], rp[:])
                st_fin[g] = hT_out

        def s3b_htr_pair(pp):
            # transpose + bias-relu for groups (2pp, 2pp+1) in one wide ACT op
            ht_ps = scr_ps_pool.tile([112, 2, NST, 128], dt.bfloat16,
                                     name="ht_ps", tag="scr")
            for half in range(2):
                hraw = st_hraw.pop(2 * pp + half)
                for stt in range(NST):
                    nc.tensor.transpose(ht_ps[:, half, stt, :], hraw[:, stt, :],
                                        ident_sb[:])
            hT = ht_pool.tile([112, 2 * GROUP], dt.bfloat16, name="hT", tag="hT0",
                              padded_shape=[128, 2 * GROUP])
            nc.scalar.activation(
                hT[:], ht_ps.rearrange("a b c d -> a (b c d)"),
                mybir.ActivationFunctionType.Relu, bias=cbias_sb[:],
            )
            st_ht[pp] = hT

        def s4_highway_pair(pp, l):
            # matmuls + relu/sigmoid per group (psum-width-bound), but the
            # three tensor-tensor ops run once per PAIR at double width
            hT = (st_ht if l == 0 else st_mid).pop(pp)
            if l == 1:
                hT_out = ht_pool.tile([128, 2 * GROUP], dt.bfloat16, name="hT_fin",
                                      tag="hT_fin", padded_shape=[128, 2 * GROUP])
                eng(kones).memset(hT_out[96:128, :], 1.0)
            rp = act_pool.tile([112, 2 * GROUP], dt.bfloat16, name="rp",
                               tag=f"rp{l}", padded_shape=[128, 2 * GROUP])
            gs = act_pool.tile([112, 2 * GROUP], dt.bfloat16, name="gs",
                               tag=f"gs{l}", padded_shape=[128, 2 * GROUP])
            tp, tg = ("p", "g") if (l == 0) == (kpgsw == 0) else ("g", "p")
            for half in range(2):
                sl = slice(half * GROUP, (half + 1) * GROUP)
                p_ps = pg_ps_pool.tile([112, GROUP], dt.float32, name="p_ps", tag=tp)
                g_ps = pg_ps_pool.tile([112, GROUP], dt.float32, name="g_ps", tag=tg)
                nc.tensor.matmul(p_ps, lhsT=hwT_sb[:, l * 224:l * 224 + 112],
                                 rhs=hT[0:112, sl], start=True, stop=True)
                nc.tensor.matmul(g_ps, lhsT=hwT_sb[:, l * 224 + 112:l * 224 + 224],
                                 rhs=hT[0:112, sl], start=True, stop=True)
                if krelu[l] == "act":
                    nc.scalar.activation(rp[:, sl], p_ps,
                                         mybir.ActivationFunctionType.Relu,
                                         bias=hbias_sb[:, 2 * l:2 * l + 1])
                else:
                    eng(krelu[l]).tensor_scalar(
                        out=rp[:, sl], in0=p_ps, scalar1=hbias_sb[:, 2 * l:2 * l + 1],
                        scalar2=0.0, op0=mybir.AluOpType.add, op1=mybir.AluOpType.max)
                nc.scalar.activation(gs[:, sl], g_ps,
                                     mybir.ActivationFunctionType.Sigmoid,
                                     bias=hbias_sb[:, 2 * l + 1:2 * l + 2])
            d = act_pool.tile([112, 2 * GROUP], dt.bfloat16, name="d", tag=f"d{l}",
                              padded_shape=[128, 2 * GROUP])
            eng(ksub[l]).tensor_sub(d[:], hT[0:112, :], rp[:])
            e = act_pool.tile([112, 2 * GROUP], dt.bfloat16, name="e", tag=f"e{l}",
                              padded_shape=[128, 2 * GROUP])
            eng(kmul[l]).tensor_mul(e[:], gs[:], d[:])
            if l == 0:
                hT_next = ht_pool.tile([112, 2 * GROUP], dt.bfloat16, name="hT_mid",
                                       tag="hT_mid", padded_shape=[128, 2 * GROUP])
                eng(kadd[l]).tensor_add(hT_next[0:112, :], e[:], rp[:])
                st_mid[pp] = hT_next
            else:
                eng(kadd[l]).tensor_add(hT_out[0:112, :], e[:], rp[:])
                st_fin[pp] = hT_out

        st_osb = {}

        def s5_proj(g):
            q0, nst = sched[g]
            if kpair:
                pp = g // 2
                hT_pair = st_fin[pp]
                if g % 2 == 1:
                    del st_fin[pp]
                hT = hT_pair[:, (g % 2) * GROUP:(g % 2 + 1) * GROUP]
            else:
                hT = st_fin.pop(g)
            osb = out_pool.tile([128, nst, OUT_DIM], dt.bfloat16, name="osb",
                                tag="osb", padded_shape=[128, NST, OUT_DIM])
            # last group: alternate copy engines so the drain chain
            # (mm -> copy -> mm -> copy ...) overlaps instead of serializing
            # on ACT
            kc = ["act", "dve", "act", "dve"] if g >= NGR - ksplit else kcopy
            for stt in range(nst):
                o_ps = scr_ps_pool.tile([128, OUT_DIM], dt.float32, name="o_ps", tag="scr")
                if "pjmm" in kabl:
                    nc.tensor.matmul(o_ps[:, 0:1], lhsT=hT[:, stt * 128:(stt + 1) * 128],
                                     rhs=pwT_sb[:, 0:1], start=True, stop=True)
                else:
                    nc.tensor.matmul(o_ps[:], lhsT=hT[:, stt * 128:(stt + 1) * 128],
                                     rhs=pwT_sb[:], start=True, stop=True)
                ce = kc[stt % len(kc)]
                if "pjcp" in kabl:
                    nc.scalar.copy(out=osb[:, stt, 0:1], in_=o_ps[:, 0:1])
                elif ce == "act":
                    nc.scalar.copy(out=osb[:, stt, :], in_=o_ps[:])
                elif ce == "dve":
                    nc.vector.tensor_copy(out=osb[:, stt, :], in_=o_ps[:])
                else:
                    nc.gpsimd.tensor_copy(out=osb[:, stt, :], in_=o_ps[:])
            st_osb[g] = osb

        st_eng = {"sp": nc.sync, "pool": nc.gpsimd, "act": nc.scalar,
                  "dve": nc.vector}[kstq]

        def s6_store(g):
            q0, nst = sched[g]
            osb = st_osb.pop(g)
            if "st" in kabl:
                st_eng.dma_start(out=outv[:, q0:q0 + 1, 0:1], in_=osb[:, 0:1, 0:1])
                return
            if g >= NGR - ksplit and nst > 1:
                # split the final store so its first half overlaps the
                # second half's psum->sbuf copies
                h = nst // 2
                st_eng.dma_start(out=outv[:, q0:q0 + h, :], in_=osb[:, 0:h])
                st_eng.dma_start(out=outv[:, q0 + h:q0 + nst, :], in_=osb[:, h:nst])
            else:
                st_eng.dma_start(out=outv[:, q0:q0 + nst, :], in_=osb[:])

        SKEWS = {
            "wide":    {"st": 8, "pj": 7, "h1": 6, "h0": 5, "tr": 4, "mp": 3, "cv": 2, "xb": 1, "ld": 0},
            "mid":     {"st": 7, "pj": 6, "h1": 5, "h0": 5, "tr": 4, "mp": 3, "cv": 2, "xb": 1, "ld": 0},
            "compact": {"st": 6, "pj": 5, "h1": 4, "h0": 4, "tr": 3, "mp": 3, "cv": 2, "xb": 1, "ld": 0},
            "c2":      {"st": 6, "pj": 5, "h1": 4, "h0": 4, "tr": 4, "mp": 3, "cv": 2, "xb": 1, "ld": 0},
            "tight":   {"st": 5, "pj": 4, "h1": 4, "h0": 4, "tr": 3, "mp": 3, "cv": 2, "xb": 1, "ld": 0},
        }[kskew]
        if kpair:
            # pair stages (tr/h0/h1) complete at odd iterations; push the
            # per-group consumers one iteration later
            SKEWS = dict(SKEWS)
            SKEWS["pj"] += 1
            SKEWS["st"] += 1
        STAGES = {
            "st": (SKEWS["st"], s6_store), "pj": (SKEWS["pj"], s5_proj),
            "mp": (SKEWS["mp"], s3_maxpool),
            "h1": (SKEWS["h1"], (lambda g: s4_highway_pair(g // 2, 1) if g % 2 else None)
                   if kpair else (lambda g: s4_highway(g, 1))),
            "h0": (SKEWS["h0"], (lambda g: s4_highway_pair(g // 2, 0) if g % 2 else None)
                   if kpair else (lambda g: s4_highway(g, 0))),
            "tr": (SKEWS["tr"], (lambda g: s3b_htr_pair(g // 2) if g % 2 else None)
                   if kpair else s3b_htr), "cv": (SKEWS["cv"], s2_conv),
            "xb": (SKEWS["xb"], s1_xbar), "ld": (SKEWS["ld"], s0_load),
        }
        ORDERS = {
            "A": ["st", "pj", "mp", "h1", "h0", "tr", "cv", "xb", "ld"],
            "B": ["st", "pj", "mp", "tr", "cv", "h1", "h0", "xb", "ld"],
            "C": ["st", "pj", "mp", "cv", "h1", "h0", "tr", "xb", "ld"],
            "D": ["st", "pj", "h1", "h0", "mp", "tr", "cv", "xb", "ld"],
            "E": ["st", "pj", "h1", "mp", "cv", "h0", "tr", "xb", "ld"],
            "F": ["st", "pj", "mp", "h1", "cv", "h0", "tr", "xb", "ld"],
            "G": ["st", "h1", "pj", "mp", "cv", "h0", "tr", "xb", "ld"],
            "H": ["st", "h1", "pj", "mp", "h0", "cv", "tr", "xb", "ld"],
            "I": ["st", "mp", "pj", "h1", "cv", "h0", "tr", "xb", "ld"],
            "J": ["st", "mp", "h1", "pj", "cv", "h0", "tr", "xb", "ld"],
            "K": ["st", "mp", "pj", "h1", "h0", "cv", "tr", "xb", "ld"],
            "L": ["st", "mp", "h1", "cv", "pj", "h0", "tr", "xb", "ld"],
            "M": ["st", "mp", "h1", "pj", "cv", "tr", "h0", "xb", "ld"],
            "N": ["st", "mp", "h1", "pj", "h0", "cv", "tr", "xb", "ld"],
            "O": ["mp", "st", "h1", "pj", "cv", "h0", "tr", "xb", "ld"],
            "V": ["st", "mp", "pj", "h0", "h1", "cv", "tr", "xb", "ld"],
            "W": ["mp", "st", "pj", "h0", "h1", "tr", "cv", "xb", "ld"],
            "P": ["st", "mp", "pj", "cv", "h0", "h1", "tr", "xb", "ld"],
            "Q": ["st", "mp", "cv", "pj", "h0", "h1", "tr", "xb", "ld"],
            "R": ["st", "mp", "pj", "h0", "h1", "cv", "tr", "xb", "ld"],
            "S": ["st", "mp", "pj", "h0", "h1", "tr", "cv", "xb", "ld"],
            "T": ["st", "mp", "cv", "h0", "h1", "pj", "tr", "xb", "ld"],
            "U": ["st", "mp", "h0", "h1", "pj", "cv", "tr", "xb", "ld"],
            "Z": ["ld", "st", "mp", "pj", "h0", "h1", "tr", "cv", "xb"],
            "Y": ["ld", "xb", "st", "mp", "pj", "h0", "h1", "tr", "cv"],
        }
        if kwarm:
            # keep PE busy from t=0 so the HAM clock gate releases before the
            # first conv group arrives (pg psum bank is unused during fill)
            warm_ps = pg_ps_pool.tile([112, GROUP], dt.float32, name="p_ps", tag="p")
            for _ in range(kwarm):
                nc.tensor.matmul(warm_ps[:, 0:64], lhsT=hwT_sb[:, 0:112],
                                 rhs=hwT_sb[:, 0:64], start=True, stop=True)

        if kedge == "1":
            base = [(0, 2), (2, 2)] + [(4 + 4 * i, 4) for i in range(6)] + [(28, 2), (30, 2)]
        elif kedge == "2":
            # drain-only: halve just the final group
            base = [(4 * i, 4) for i in range(NG - 1)] + [(28, 2), (30, 2)]
        else:
            base = [(NST * i, NST) for i in range(NG)]
        sched = []
        for r in range(reps):
            sched.extend(base)
        NGR = len(sched)

        def emit_fillers(n):
            # independent junk matmuls over wbig into the (yet unused) p bank:
            # keeps the PE clock hot across fill-phase gaps
            warm_ps = pg_ps_pool.tile([112, GROUP], dt.float32, name="p_ps", tag="p")
            for _ in range(n):
                nc.tensor.matmul(warm_ps[:, 0:128], lhsT=wbig_sb[0:128, 0, 0:112],
                                 rhs=wbig_sb[:, 0, 0:128], start=True, stop=True)

        if kv5:
            # v5 driver: whole-core SBUF residency for the input stream. All
            # loads + xbar transposes are issued upfront (xg/xt pools sized to
            # NGR bufs), so the steady-state loop carries only compute stages
            # and stores — no DMA latency inside the per-iteration chain.
            if kpre:
                load_early_consts()
            for g in range(NGR):
                s0_load(g)
            for g in range(NGR):
                s1_xbar(g)
            load_late_consts()
            if kfill:
                emit_fillers(kfill)
            FN5 = {"cv": s2_conv, "mp": s3_maxpool, "tr": s3b_htr,
                   "h0": lambda g: s4_highway(g, 0),
                   "h1": lambda g: s4_highway(g, 1),
                   "pj": s5_proj, "st": s6_store}
            SK5 = {}
            for ent in kv5sk.split(","):
                k, v = ent.split(":")
                SK5[k] = int(v)
            ORD5 = kv5ord.split(",")
            for it in range(NGR + max(SK5.values()) + 1):
                for key in ORD5:
                    g = it - SK5[key]
                    if 0 <= g < NGR:
                        FN5[key](g)
        else:
            if kpre:
                load_early_consts()
            if kfillpre:
                emit_fillers(kfillpre)
            for it in range(NGR + 8):
                if it == 2:
                    load_late_consts()
                for key in ORDERS[kord]:
                    skew, fn = STAGES[key]
                    g = it - skew
                    if 0 <= g < NGR:
                        fn(g)
                if kfill and it < 4:
                    emit_fillers(kfill if it < 2 else kfill // 2)

    nc.compile()
    return nc


def _prep_weights(inputs):
    W = np.zeros((FEAT, CONV_COLS), np.float32)
    cb = np.zeros(LAST_DIM, np.float32)
    off = 0
    offc = 0
    for p_pad, flist in SEGS:
        for w, c in flist:
            i = w  # filter index == width for this problem
            p_i = BYTE_LEN - w + 1
            cw = np.asarray(inputs[f"conv_w{i}"], np.float32)  # [c, EMB, w]
            for p in range(p_pad):
                sp = p if p < p_i else 0  # duplicate position 0 as padding
                for k in range(w):
                    byte = sp + k
                    W[byte * EMB:(byte + 1) * EMB,
                      off + p:off + c * p_pad:p_pad] = cw[:, :, k].T
            cb[offc:offc + c] = np.asarray(inputs[f"conv_b{i}"], np.float32)
            off += c * p_pad
            offc += c
    wbig = np.ascontiguousarray(
        W.reshape(4, 128, CONV_COLS).transpose(1, 0, 2).reshape(128, 4 * CONV_COLS)
    ).astype(bf16)
    hwT = np.concatenate([np.asarray(inputs["hw_w1"], np.float32).T,
                          np.asarray(inputs["hw_w2"], np.float32).T], 1)
    hwT = np.ascontiguousarray(hwT).astype(bf16)  # [112, 448]
    pwT = np.zeros((128, 512), np.float32)
    pwT[:112] = np.asarray(inputs["proj_w"], np.float32).T
    pwT[112] = np.asarray(inputs["proj_b"], np.float32)
    pwT = np.ascontiguousarray(pwT).astype(bf16)
    hb1 = np.asarray(inputs["hw_b1"], np.float32)
    hb2 = np.asarray(inputs["hw_b2"], np.float32)
    hbias = np.stack([hb1[:112], hb1[112:], hb2[:112], hb2[112:]], 1)  # [112, 4]
    hbias = np.ascontiguousarray(hbias)
    return wbig, hwT, pwT, cb.reshape(112, 1), hbias


def _in_maps(inputs):
    wbig, hwT, pwT, cb, hbias = _prep_weights(inputs)
    ident = np.eye(128, dtype=bf16)
    feats = np.ascontiguousarray(
        np.asarray(inputs["features"], np.float32).reshape(B * T, FEAT)
    )
    return [{
        "features": feats[c * S_PER_CORE:(c + 1) * S_PER_CORE],
        "wbig": wbig, "hwT": hwT, "pwT": pwT, "cbias": cb, "hbias": hbias,
        "ident": ident,
    } for c in range(N_CORES)]


def kernel(**inputs) -> np.ndarray:
    from concourse.bass_utils import run_bass_kernel_spmd

    if "nc" not in _cache:
        _cache["nc"] = _build()
    nc = _cache["nc"]

    in_maps = _in_maps(inputs)
    res = run_bass_kernel_spmd(nc, in_maps, core_ids=list(range(N_CORES)))
    out = np.concatenate([res.results[c]["out"] for c in range(N_CORES)], 0)
    return np.ascontiguousarray(out.reshape(B, T, OUT_DIM)).astype(np.float32)



# revision 30
# speedup vs baseline: 1.1036x; 1.0171x over previous
# Trainium2 Bass kernel for ByteCombineCNN — software-pipelined rewrite (v4).
#
# Same math as the baseline kernel (conv-as-dense-matmul + segmented maxpool,
# highway with ACT per-partition bias, projection bias via ones-row), emitted
# stage-major so no engine's in-order queue couples a group's late stages to
# the next group's early stages:
#   S0 load (SWDGE cast f32->bf16)       Pool ring       g = it
#   S1 input dma-xbar transpose          SP HWDGE        g = it-1
#   S2 conv matmuls (16)                 PE              g = it-2
#   S3 segmented maxpool (14 reduces)    DVE (+Pool)     g = it-3
#   S3b h transpose (PE) + bias-relu     PE + ACT        g = it-4
#   S4a highway layer 0                  PE + ACT/DVE    g = it-5
#   S4b highway layer 1                  PE + ACT/DVE/Pool  g = it-6
#   S5 projection + per-subtile store    PE + SP HWDGE   g = it-7
# The projection output is DMA'd directly from PSUM to HBM in f32 (no
# psum->sbuf copies, no output staging buffer, no host-side cast).
# PSUM: conv 2x2 banks + highway p/g 2 + {ht_ps,o_ps} shared 2 = 8 banks.
import numpy as np
import ml_dtypes

bf16 = ml_dtypes.bfloat16

B, T, BYTE_LEN, EMB = 8, 4096, 8, 64
FILTERS = [(1, 4), (2, 8), (3, 12), (4, 16), (5, 20), (6, 24), (7, 28)]
NPOS = [BYTE_LEN - w + 1 for w, _ in FILTERS]
# Filters merged into segments with a common (padded) position count so the
# segmented maxpool needs one reduce per segment instead of one per filter.
# Padded positions duplicate position 0 (max(a,a,...) == max(a,...)).
SEGS = [(8, [(1, 4), (2, 8)]), (6, [(3, 12), (4, 16)]),
        (4, [(5, 20), (6, 24)]), (2, [(7, 28)])]
LAST_DIM = 112
OUT_DIM = 512
FEAT = BYTE_LEN * EMB          # 512
CONV_COLS = sum(pp * sum(c for _, c in fl) for pp, fl in SEGS)  # 496
N_CORES = 8
S_PER_CORE = B * T // N_CORES  # 4096
import os as _os
GROUP = int(_os.environ.get("KGRP", "512")) if _os.environ.get("KDEV", "0") == "1" else 512
NG = S_PER_CORE // GROUP       # 8
NST = GROUP // 128             # 4

_cache = {}


def _build(reps=1):
    import os
    import concourse.mybir as mybir
    import concourse.tile as tile
    from concourse import bacc
    from contextlib import ExitStack

    dt = mybir.dt
    nc = bacc.Bacc("TRN2", target_bir_lowering=False, debug=False)

    feat = nc.dram_tensor("features", [S_PER_CORE, FEAT], dt.float32, kind="ExternalInput").ap()
    wbig_d = nc.dram_tensor("wbig", [128, 4 * CONV_COLS], dt.bfloat16, kind="ExternalInput").ap()
    hwT_d = nc.dram_tensor("hwT", [112, 448], dt.bfloat16, kind="ExternalInput").ap()
    pwT_d = nc.dram_tensor("pwT", [128, 512], dt.bfloat16, kind="ExternalInput").ap()
    cbias_d = nc.dram_tensor("cbias", [112, 1], dt.float32, kind="ExternalInput").ap()
    hbias_d = nc.dram_tensor("hbias", [112, 4], dt.float32, kind="ExternalInput").ap()
    ident_d = nc.dram_tensor("ident", [128, 128], dt.bfloat16, kind="ExternalInput").ap()
    outp = nc.dram_tensor("out", [S_PER_CORE, OUT_DIM], dt.bfloat16, kind="ExternalOutput").ap()

    featv = feat.rearrange("(q p) f -> p q f", p=128)     # [128, 32 subtiles, 512]
    outv = outp.rearrange("(q p) o -> p q o", p=128)      # [128, 32 subtiles, 512]

    dev = os.environ.get("KDEV", "0") == "1"

    def _env(name, default):
        return os.environ.get(name, default) if dev else default

    def eng(name):
        return {"dve": nc.vector, "pool": nc.gpsimd, "act": nc.scalar}[name]

    kabl = set(_env("KABL", "").split(",")) - {""}  # ablate stages (sim probe)
    kv5 = _env("KV5", "0") == "1"                  # upfront input stream driver
    kv5sk = _env("KV5SK", "cv:0,mp:1,tr:1,h0:2,h1:2,pj:3,st:3")
    kv5ord = _env("KV5ORD", "st,pj,h1,h0,tr,mp,cv")
    kstq = _env("KSTQ", "sp")                      # store queue: sp|pool|act|dve
    kpre = int(_env("KPRE", "1"))                  # preload ACT tables at t=0
    kfill = int(_env("KFILL", "0"))                # filler matmuls per fill iter
    kfillpre = int(_env("KFILLPRE", "70"))         # fillers emitted before loop
    kpair = _env("KPAIR", "0") == "1"              # pair groups in tr/hw stages
    kld0 = int(_env("KLD0", "0"))                  # split ld/xb for first n groups
    kfstat = int(_env("KFSTAT", "0"))              # static hT_fin ring (ones rows
                                                   # memset once at startup)
    ksub = _env("KSUB", "dve,dve").split(",")     # per-layer sub engine
    kmul = _env("KMUL", "dve,dve").split(",")      # per-layer mul engine
    kadd = _env("KADD", "dve,dve").split(",")     # per-layer add engine
    krelu = _env("KRELU", "act,act").split(",")    # per-layer relu engine
    kones = _env("KONES", "dve")                   # ones-rows memset engine
    kcopy = _env("KCOPY", "act,act,act,dve").split(",")  # per-st proj copy engine
    kpoolred = int(_env("KPOOLRED", "0"))          # first n filters' maxpool on Pool
    kord = _env("KORD", "S")                       # per-iteration stage emission order
    ktr = int(_env("KTR", "1"))                    # input xbar transposes per group
    kwarm = int(_env("KWARM", "0"))                # PE warm-up matmuls during fill
    kedge = _env("KEDGE", "0")                     # edge-group mode: 0/1/2
    ksplit = int(_env("KSPLIT", "2"))              # last n groups: split store
    kcv1 = _env("KCV1", "0") == "1"                # single 4-bank conv psum tile
    kdrelu = int(_env("KDRELU", "0"))              # last n groups: relu on DVE
    kpgsw = int(_env("KPGSW", "0"))                # swap p/g psum tags on layer 1
    kgs1 = _env("KGS1", "0") == "1"                # emit sigmoid before relu
    kmpi = _env("KMPI", "0") == "1"                # interleave maxpool reduces across pr
    kmpr = _env("KMPR", "0") == "1"                # reverse maxpool segment order
    kskew = _env("KSKEW", "compact")                  # pipeline skew table
    kbufs = {k: int(_env("KB_" + k, v)) for k, v in
             [("xg", "2"), ("xt", "5"), ("hraw", "3"), ("ht", "5"), ("act", "5")]}
    if kv5:
        # upfront input streaming needs every group's staging + transposed
        # tile resident at once
        kbufs["xg"] = NG
        kbufs["xt"] = NG

    with tile.TileContext(nc) as tc, ExitStack() as ctx:
        const = ctx.enter_context(tc.tile_pool(name="const", bufs=1))
        wbig_sb = const.tile([128, 4, CONV_COLS], dt.bfloat16, name="wbig_sb")
        nc.sync.dma_start(out=wbig_sb[:], in_=wbig_d.rearrange("p (k c) -> p k c", k=4))
        hwT_sb = const.tile([112, 448], dt.bfloat16, name="hwT_sb")
        pwT_sb = const.tile([128, 512], dt.bfloat16, name="pwT_sb")
        cbias_sb = const.tile([112, 1], dt.float32, name="cbias_sb")
        hbias_sb = const.tile([112, 4], dt.float32, name="hbias_sb")
        ident_sb = const.tile([128, 128], dt.bfloat16, name="ident_sb")

        pre_sb = const.tile([112, 1], dt.bfloat16, name="pre_sb") if kpre else None

        fin_bufs = []
        if kfstat:
            # manually-rotated hT_fin ring: the proj-bias ones rows (112:128)
            # are written once here and never touched again — h1 only rewrites
            # rows 0:112 — saving one memset per group on the hot engines
            for i in range(kfstat):
                b = const.tile([128, GROUP], dt.bfloat16, name=f"hTfin{i}")
                eng(kones).memset(b[96:128, :], 1.0)
                fin_bufs.append(b)

        def load_early_consts():
            # tiny biases first so the ACT-table preload dummies have real
            # operands; the big weights stay at iteration 2.
            nc.scalar.dma_start(out=cbias_sb[:], in_=cbias_d)
            nc.scalar.dma_start(out=hbias_sb[:], in_=hbias_d)
            # trigger every ACT function-set load while the pipe is filling
            nc.scalar.activation(pre_sb[:], cbias_sb[:],
                                 mybir.ActivationFunctionType.Copy)
            nc.scalar.activation(pre_sb[:], cbias_sb[:],
                                 mybir.ActivationFunctionType.Relu,
                                 bias=cbias_sb[:])
            nc.scalar.activation(pre_sb[:], cbias_sb[:],
                                 mybir.ActivationFunctionType.Sigmoid,
                                 bias=cbias_sb[:])

        def load_late_consts():
            # emitted at iteration 2 and on the ACT HWDGE ring so the early
            # input transposes own the SP ring; first consumers run at
            # iteration 3+.
            nc.scalar.dma_start(out=hwT_sb[:], in_=hwT_d)
            nc.scalar.dma_start(out=pwT_sb[:], in_=pwT_d)
            if not kpre:
                nc.scalar.dma_start(out=cbias_sb[:], in_=cbias_d)
                nc.scalar.dma_start(out=hbias_sb[:], in_=hbias_d)
            nc.scalar.dma_start(out=ident_sb[:], in_=ident_d)

        xg_pool = ctx.enter_context(tc.tile_pool(name="xg", bufs=kbufs["xg"]))
        xt_pool = ctx.enter_context(tc.tile_pool(name="xt", bufs=kbufs["xt"]))
        conv_ps_pool = ctx.enter_context(tc.tile_pool(
            name="conv_ps", bufs=1 if kcv1 else 2, space="PSUM"))
        hraw_pool = ctx.enter_context(tc.tile_pool(name="hraw", bufs=kbufs["hraw"]))
        ht_pool = ctx.enter_context(tc.tile_pool(name="ht", bufs=kbufs["ht"]))
        act_pool = ctx.enter_context(tc.tile_pool(name="act", bufs=kbufs["act"]))
        pg_ps_pool = ctx.enter_context(tc.tile_pool(name="pg_ps", bufs=1, space="PSUM"))
        scr_ps_pool = ctx.enter_context(tc.tile_pool(name="scr_ps", bufs=2, space="PSUM"))
        out_pool = ctx.enter_context(tc.tile_pool(name="outsb", bufs=3))

        st_xg = {}
        st_xt = {}
        st_conv = {}
        st_hraw = {}
        st_ht = {}          # (g) -> hT after relu (input of layer 0)
        st_mid = {}         # (g) -> hT after layer 0
        st_fin = {}         # (g) -> hT_fin after layer 1

        def s0_load(g):
            q0, nst = sched[g]
            xg = xg_pool.tile([128, nst * FEAT], dt.bfloat16, name="xg", tag="xg")
            if "ld" in kabl:
                nc.gpsimd.dma_start(out=xg[:, 0:1], in_=featv[:, q0:q0 + 1, 0:1])
            elif g < kld0 and nst > 1:
                # split the fill-critical first loads so the first conv tile's
                # data lands ~1.5us earlier
                h = nst // 2
                nc.gpsimd.dma_start(out=xg[:, 0:h * FEAT],
                                    in_=featv[:, q0:q0 + h, :])
                nc.gpsimd.dma_start(out=xg[:, h * FEAT:nst * FEAT],
                                    in_=featv[:, q0 + h:q0 + nst, :])
            else:
                nc.gpsimd.dma_start(out=xg[:], in_=featv[:, q0:q0 + nst, :])
            st_xg[g] = xg

        def s1_xbar(g):
            q0, nst = sched[g]
            xt = xt_pool.tile([128, nst, 4, 128], dt.bfloat16, name="xt", tag="xt",
                              padded_shape=[128, NST, 4, 128])
            xg = st_xg.pop(g)
            if "xb" in kabl:
                nc.sync.dma_start_transpose(
                    out=xt[0:64, 0, 0, :], in_=xg[:, 0:64])
            elif g < kld0 and nst > 1:
                h = nst // 2
                nc.sync.dma_start_transpose(out=xt[:, 0:h], in_=xg[:, 0:h * FEAT])
                nc.sync.dma_start_transpose(out=xt[:, h:nst],
                                            in_=xg[:, h * FEAT:nst * FEAT])
            else:
                nc.sync.dma_start_transpose(out=xt[:], in_=xg[:])
            st_xt[g] = xt

        def s2_conv(g):
            q0, nst = sched[g]
            xt = st_xt.pop(g)
            if kcv1:
                conv_ps = conv_ps_pool.tile([128, nst, 512], dt.float32,
                                            name="conv_ps", tag="conv_ps",
                                            padded_shape=[128, NST, 512])
                for stt in range(nst):
                    for kc in range(4):
                        if "cv" in kabl: continue
                        nc.tensor.matmul(
                            conv_ps[:, stt, 0:CONV_COLS],
                            lhsT=xt[:, stt, kc, :],
                            rhs=wbig_sb[:, kc, :],
                            start=(kc == 0),
                            stop=(kc == 3),
                        )
                st_conv[g] = [conv_ps]
                return
            tiles = []
            pw = min(2, nst)  # subtiles per conv psum tile
            for pr in range(nst // pw):
                conv_ps = conv_ps_pool.tile([128, pw, 512], dt.float32, name="conv_ps")
                for sub in range(pw):
                    stt = pr * pw + sub
                    for kc in range(4):
                        if "cv" in kabl:
                            if kc == 0:
                                nc.tensor.matmul(
                                    conv_ps[:, sub, 0:1],
                                    lhsT=xt[:, stt, 0, :],
                                    rhs=wbig_sb[:, 0, 0:1],
                                    start=True, stop=True)
                            continue
                        nc.tensor.matmul(
                            conv_ps[:, sub, 0:CONV_COLS],
                            lhsT=xt[:, stt, kc, :],
                            rhs=wbig_sb[:, kc, :],
                            start=(kc == 0),
                            stop=(kc == 3),
                        )
                tiles.append(conv_ps)
            st_conv[g] = tiles

        def s3_maxpool(g):
            q0, nst = sched[g]
            tiles = st_conv.pop(g)
            hraw = hraw_pool.tile([128, nst, LAST_DIM], dt.bfloat16, name="hraw",
                                  tag="hraw", padded_shape=[128, NST, LAST_DIM])
            bw = nst // len(tiles)  # subtiles covered per conv psum tile
            calls = []
            for pr, conv_ps in enumerate(tiles):
                off = 0
                offc = 0
                for si, (p_pad, flist) in enumerate(SEGS):
                    cseg = sum(c for _, c in flist)
                    calls.append((pr, si, conv_ps, off, offc, cseg, p_pad))
                    off += cseg * p_pad
                    offc += cseg
            if kmpi:
                calls.sort(key=lambda t: (t[1], t[0]))  # segment-major interleave
            if kmpr:
                calls.sort(key=lambda t: (t[0], -t[1]))
            for pr, si, conv_ps, off, offc, cseg, p_pad in calls:
                if "mp" in kabl:
                    p_pad = 1
                seg = conv_ps[:, 0:bw, off:off + cseg * p_pad].rearrange(
                    "a b (cc p) -> a b cc p", p=p_pad
                )
                nc.vector.tensor_reduce(
                    out=hraw[:, pr * bw:(pr + 1) * bw, offc:offc + cseg],
                    in_=seg,
                    axis=mybir.AxisListType.X,
                    op=mybir.AluOpType.max,
                )
            st_hraw[g] = hraw

        def s3b_htr(g):
            q0, nst = sched[g]
            ht_ps = scr_ps_pool.tile([112, NST, 128], dt.bfloat16, name="ht_ps", tag="scr")
            hraw = st_hraw.pop(g)
            for stt in range(nst):
                if "trp" in kabl: continue
                nc.tensor.transpose(ht_ps[:, stt, :], hraw[:, stt, :], ident_sb[:])
            hT = ht_pool.tile([112, nst * 128], dt.bfloat16, name="hT", tag="hT0",
                              padded_shape=[128, GROUP])
            if "tra" in kabl:
                nc.scalar.activation(
                    hT[:, 0:1], ht_ps[:, 0, 0:1],
                    mybir.ActivationFunctionType.Relu, bias=cbias_sb[:],
                )
            else:
                nc.scalar.activation(
                    hT[:], ht_ps[:, 0:nst].rearrange("a b c -> a (b c)"),
                    mybir.ActivationFunctionType.Relu, bias=cbias_sb[:],
                )
            st_ht[g] = hT

        def s4_highway(g, l):
            q0, nst = sched[g]
            W = nst * 128
            hT = (st_ht if l == 0 else st_mid).pop(g)
            if l == 1:
                # allocate the output tile up-front so the ones-rows memset
                # runs off the critical l1 chain
                if kfstat:
                    hT_out = fin_bufs[g % kfstat]
                else:
                    hT_out = ht_pool.tile([128, W], dt.bfloat16, name="hT_fin",
                                          tag="hT_fin", padded_shape=[128, GROUP])
                    eng(kones).memset(hT_out[96:128, :], 1.0)
            tp, tg = ("p", "g") if (l == 0) == (kpgsw == 0) else ("g", "p")
            p_ps = pg_ps_pool.tile([112, GROUP], dt.float32, name="p_ps", tag=tp)
            g_ps = pg_ps_pool.tile([112, GROUP], dt.float32, name="g_ps", tag=tg)
            p_ps = p_ps[:, 0:W]
            g_ps = g_ps[:, 0:W]
            if "hwmm" in kabl:
                nc.tensor.matmul(p_ps[:, 0:1], lhsT=hwT_sb[:, l * 224:l * 224 + 112],
                                 rhs=hT[0:112, 0:1], start=True, stop=True)
                nc.tensor.matmul(g_ps[:, 0:1], lhsT=hwT_sb[:, l * 224 + 112:l * 224 + 224],
                                 rhs=hT[0:112, 0:1], start=True, stop=True)
            else:
                nc.tensor.matmul(p_ps, lhsT=hwT_sb[:, l * 224:l * 224 + 112],
                                 rhs=hT[0:112, :], start=True, stop=True)
                nc.tensor.matmul(g_ps, lhsT=hwT_sb[:, l * 224 + 112:l * 224 + 224],
                                 rhs=hT[0:112, :], start=True, stop=True)
            rp = act_pool.tile([112, W], dt.bfloat16, name="rp", tag=f"rp{l}",
                               padded_shape=[128, GROUP])
            gs = act_pool.tile([112, W], dt.bfloat16, name="gs", tag=f"gs{l}",
                               padded_shape=[128, GROUP])
            kr = krelu[l] if g < NGR - kdrelu else "dve"

            def emit_rp():
                if "hwrelu" in kabl:
                    nc.scalar.activation(rp[:, 0:1], p_ps[:, 0:1],
                                         mybir.ActivationFunctionType.Relu,
                                         bias=hbias_sb[:, 2 * l:2 * l + 1])
                    return
                if kr == "act":
                    nc.scalar.activation(rp[:], p_ps, mybir.ActivationFunctionType.Relu,
                                         bias=hbias_sb[:, 2 * l:2 * l + 1])
                else:
                    eng(kr).tensor_scalar(
                        out=rp[:], in0=p_ps, scalar1=hbias_sb[:, 2 * l:2 * l + 1],
                        scalar2=0.0, op0=mybir.AluOpType.add, op1=mybir.AluOpType.max)

            def emit_gs():
                if "hwsig" in kabl:
                    nc.scalar.activation(gs[:, 0:1], g_ps[:, 0:1],
                                         mybir.ActivationFunctionType.Sigmoid,
                                         bias=hbias_sb[:, 2 * l + 1:2 * l + 2])
                    return
                nc.scalar.activation(gs[:], g_ps, mybir.ActivationFunctionType.Sigmoid,
                                     bias=hbias_sb[:, 2 * l + 1:2 * l + 2])

            if kgs1:
                emit_gs()
                emit_rp()
            else:
                emit_rp()
                emit_gs()
            d = act_pool.tile([112, W], dt.bfloat16, name="d", tag=f"d{l}",
                              padded_shape=[128, GROUP])
            if "hwtt" in kabl:
                eng(ksub[l]).tensor_sub(d[:, 0:1], hT[0:112, 0:1], rp[:, 0:1])
            else:
                eng(ksub[l]).tensor_sub(d[:], hT[0:112, :], rp[:])
            e = act_pool.tile([112, W], dt.bfloat16, name="e", tag=f"e{l}",
                              padded_shape=[128, GROUP])
            if "hwtt" in kabl:
                eng(kmul[l]).tensor_mul(e[:, 0:1], gs[:, 0:1], d[:, 0:1])
            else:
                eng(kmul[l]).tensor_mul(e[:], gs[:], d[:])
            if l == 0:
                hT_next = ht_pool.tile([112, W], dt.bfloat16, name="hT_mid",
                                       tag="hT_mid", padded_shape=[128, GROUP])
                if "hwtt" in kabl:
                    eng(kadd[l]).tensor_add(hT_next[0:112, 0:1], e[:, 0:1], rp[:, 0:1])
                else:
                    eng(kadd[l]).tensor_add(hT_next[0:112, :], e[:], rp[:])
                st_mid[g] = hT_next
            else:
                if "hwtt" in kabl:
                    eng(kadd[l]).tensor_add(hT_out[0:112, 0:1], e[:, 0:1], rp[:, 0:1])
                else:
                    eng(kadd[l]).tensor_add(hT_out[0:112, :], e[:], rp[:])
                st_fin[g] = hT_out

        def s3b_htr_pair(pp):
            # transpose + bias-relu for groups (2pp, 2pp+1) in one wide ACT op
            ht_ps = scr_ps_pool.tile([112, 2, NST, 128], dt.bfloat16,
                                     name="ht_ps", tag="scr")
            for half in range(2):
                hraw = st_hraw.pop(2 * pp + half)
                for stt in range(NST):
                    nc.tensor.transpose(ht_ps[:, half, stt, :], hraw[:, stt, :],
                                        ident_sb[:])
            hT = ht_pool.tile([112, 2 * GROUP], dt.bfloat16, name="hT", tag="hT0",
                              padded_shape=[128, 2 * GROUP])
            nc.scalar.activation(
                hT[:], ht_ps.rearrange("a b c d -> a (b c d)"),
                mybir.ActivationFunctionType.Relu, bias=cbias_sb[:],
            )
            st_ht[pp] = hT

        def s4_highway_pair(pp, l):
            # matmuls + relu/sigmoid per group (psum-width-bound), but the
            # three tensor-tensor ops run once per PAIR at double width
            hT = (st_ht if l == 0 else st_mid).pop(pp)
            if l == 1:
                hT_out = ht_pool.tile([128, 2 * GROUP], dt.bfloat16, name="hT_fin",
                                      tag="hT_fin", padded_shape=[128, 2 * GROUP])
                eng(kones).memset(hT_out[96:128, :], 1.0)
            rp = act_pool.tile([112, 2 * GROUP], dt.bfloat16, name="rp",
                               tag=f"rp{l}", padded_shape=[128, 2 * GROUP])
            gs = act_pool.tile([112, 2 * GROUP], dt.bfloat16, name="gs",
                               tag=f"gs{l}", padded_shape=[128, 2 * GROUP])
            tp, tg = ("p", "g") if (l == 0) == (kpgsw == 0) else ("g", "p")
            for half in range(2):
                sl = slice(half * GROUP, (half + 1) * GROUP)
                p_ps = pg_ps_pool.tile([112, GROUP], dt.float32, name="p_ps", tag=tp)
                g_ps = pg_ps_pool.tile([112, GROUP], dt.float32, name="g_ps", tag=tg)
                nc.tensor.matmul(p_ps, lhsT=hwT_sb[:, l * 224:l * 224 + 112],
                                 rhs=hT[0:112, sl], start=True, stop=True)
                nc.tensor.matmul(g_ps, lhsT=hwT_sb[:, l * 224 + 112:l * 224 + 224],
                                 rhs=hT[0:112, sl], start=True, stop=True)
                if krelu[l] == "act":
                    nc.scalar.activation(rp[:, sl], p_ps,
                                         mybir.ActivationFunctionType.Relu,
                                         bias=hbias_sb[:, 2 * l:2 * l + 1])
                else:
                    eng(krelu[l]).tensor_scalar(
                        out=rp[:, sl], in0=p_ps, scalar1=hbias_sb[:, 2 * l:2 * l + 1],
                        scalar2=0.0, op0=mybir.AluOpType.add, op1=mybir.AluOpType.max)
                nc.scalar.activation(gs[:, sl], g_ps,
                                     mybir.ActivationFunctionType.Sigmoid,
                                     bias=hbias_sb[:, 2 * l + 1:2 * l + 2])
            d = act_pool.tile([112, 2 * GROUP], dt.bfloat16, name="d", tag=f"d{l}",
                              padded_shape=[128, 2 * GROUP])
            eng(ksub[l]).tensor_sub(d[:], hT[0:112, :], rp[:])
            e = act_pool.tile([112, 2 * GROUP], dt.bfloat16, name="e", tag=f"e{l}",
                              padded_shape=[128, 2 * GROUP])
            eng(kmul[l]).tensor_mul(e[:], gs[:], d[:])
            if l == 0:
                hT_next = ht_pool.tile([112, 2 * GROUP], dt.bfloat16, name="hT_mid",
                                       tag="hT_mid", padded_shape=[128, 2 * GROUP])
                eng(kadd[l]).tensor_add(hT_next[0:112, :], e[:], rp[:])
                st_mid[pp] = hT_next
            else:
                eng(kadd[l]).tensor_add(hT_out[0:112, :], e[:], rp[:])
                st_fin[pp] = hT_out

        st_osb = {}

        def s5_proj(g):
            q0, nst = sched[g]
            if kpair:
                pp = g // 2
                hT_pair = st_fin[pp]
                if g % 2 == 1:
                    del st_fin[pp]
                hT = hT_pair[:, (g % 2) * GROUP:(g % 2 + 1) * GROUP]
            else:
                hT = st_fin.pop(g)
            osb = out_pool.tile([128, nst, OUT_DIM], dt.bfloat16, name="osb",
                                tag="osb", padded_shape=[128, NST, OUT_DIM])
            # last group: alternate copy engines so the drain chain
            # (mm -> copy -> mm -> copy ...) overlaps instead of serializing
            # on ACT
            kc = ["act", "dve", "act", "dve"] if g >= NGR - ksplit else kcopy
            for stt in range(nst):
                o_ps = scr_ps_pool.tile([128, OUT_DIM], dt.float32, name="o_ps", tag="scr")
                if "pjmm" in kabl:
                    nc.tensor.matmul(o_ps[:, 0:1], lhsT=hT[:, stt * 128:(stt + 1) * 128],
                                     rhs=pwT_sb[:, 0:1], start=True, stop=True)
                else:
                    nc.tensor.matmul(o_ps[:], lhsT=hT[:, stt * 128:(stt + 1) * 128],
                                     rhs=pwT_sb[:], start=True, stop=True)
                ce = kc[stt % len(kc)]
                if "pjcp" in kabl:
                    nc.scalar.copy(out=osb[:, stt, 0:1], in_=o_ps[:, 0:1])
                elif ce == "act":
                    nc.scalar.copy(out=osb[:, stt, :], in_=o_ps[:])
                elif ce == "dve":
                    nc.vector.tensor_copy(out=osb[:, stt, :], in_=o_ps[:])
                else:
                    nc.gpsimd.tensor_copy(out=osb[:, stt, :], in_=o_ps[:])
            st_osb[g] = osb

        st_eng = {"sp": nc.sync, "pool": nc.gpsimd, "act": nc.scalar,
                  "dve": nc.vector}[kstq]

        def s6_store(g):
            q0, nst = sched[g]
            osb = st_osb.pop(g)
            if "st" in kabl:
                st_eng.dma_start(out=outv[:, q0:q0 + 1, 0:1], in_=osb[:, 0:1, 0:1])
                return
            if g >= NGR - ksplit and nst > 1:
                # split the final store so its first half overlaps the
                # second half's psum->sbuf copies
                h = nst // 2
                st_eng.dma_start(out=outv[:, q0:q0 + h, :], in_=osb[:, 0:h])
                st_eng.dma_start(out=outv[:, q0 + h:q0 + nst, :], in_=osb[:, h:nst])
            else:
                st_eng.dma_start(out=outv[:, q0:q0 + nst, :], in_=osb[:])

        SKEWS = {
            "wide":    {"st": 8, "pj": 7, "h1": 6, "h0": 5, "tr": 4, "mp": 3, "cv": 2, "xb": 1, "ld": 0},
            "mid":     {"st": 7, "pj": 6, "h1": 5, "h0": 5, "tr": 4, "mp": 3, "cv": 2, "xb": 1, "ld": 0},
            "compact": {"st": 6, "pj": 5, "h1": 4, "h0": 4, "tr": 3, "mp": 3, "cv": 2, "xb": 1, "ld": 0},
            "c2":      {"st": 6, "pj": 5, "h1": 4, "h0": 4, "tr": 4, "mp": 3, "cv": 2, "xb": 1, "ld": 0},
            "tight":   {"st": 5, "pj": 4, "h1": 4, "h0": 4, "tr": 3, "mp": 3, "cv": 2, "xb": 1, "ld": 0},
        }[kskew]
        if kpair:
            # pair stages (tr/h0/h1) complete at odd iterations; push the
            # per-group consumers one iteration later
            SKEWS = dict(SKEWS)
            SKEWS["pj"] += 1
            SKEWS["st"] += 1
        STAGES = {
            "st": (SKEWS["st"], s6_store), "pj": (SKEWS["pj"], s5_proj),
            "mp": (SKEWS["mp"], s3_maxpool),
            "h1": (SKEWS["h1"], (lambda g: s4_highway_pair(g // 2, 1) if g % 2 else None)
                   if kpair else (lambda g: s4_highway(g, 1))),
            "h0": (SKEWS["h0"], (lambda g: s4_highway_pair(g // 2, 0) if g % 2 else None)
                   if kpair else (lambda g: s4_highway(g, 0))),
            "tr": (SKEWS["tr"], (lambda g: s3b_htr_pair(g // 2) if g % 2 else None)
                   if kpair else s3b_htr), "cv": (SKEWS["cv"], s2_conv),
            "xb": (SKEWS["xb"], s1_xbar), "ld": (SKEWS["ld"], s0_load),
        }
        ORDERS = {
            "A": ["st", "pj", "mp", "h1", "h0", "tr", "cv", "xb", "ld"],
            "B": ["st", "pj", "mp", "tr", "cv", "h1", "h0", "xb", "ld"],
            "C": ["st", "pj", "mp", "cv", "h1", "h0", "tr", "xb", "ld"],
            "D": ["st", "pj", "h1", "h0", "mp", "tr", "cv", "xb", "ld"],
            "E": ["st", "pj", "h1", "mp", "cv", "h0", "tr", "xb", "ld"],
            "F": ["st", "pj", "mp", "h1", "cv", "h0", "tr", "xb", "ld"],
            "G": ["st", "h1", "pj", "mp", "cv", "h0", "tr", "xb", "ld"],
            "H": ["st", "h1", "pj", "mp", "h0", "cv", "tr", "xb", "ld"],
            "I": ["st", "mp", "pj", "h1", "cv", "h0", "tr", "xb", "ld"],
            "J": ["st", "mp", "h1", "pj", "cv", "h0", "tr", "xb", "ld"],
            "K": ["st", "mp", "pj", "h1", "h0", "cv", "tr", "xb", "ld"],
            "L": ["st", "mp", "h1", "cv", "pj", "h0", "tr", "xb", "ld"],
            "M": ["st", "mp", "h1", "pj", "cv", "tr", "h0", "xb", "ld"],
            "N": ["st", "mp", "h1", "pj", "h0", "cv", "tr", "xb", "ld"],
            "O": ["mp", "st", "h1", "pj", "cv", "h0", "tr", "xb", "ld"],
            "V": ["st", "mp", "pj", "h0", "h1", "cv", "tr", "xb", "ld"],
            "W": ["mp", "st", "pj", "h0", "h1", "tr", "cv", "xb", "ld"],
            "P": ["st", "mp", "pj", "cv", "h0", "h1", "tr", "xb", "ld"],
            "Q": ["st", "mp", "cv", "pj", "h0", "h1", "tr", "xb", "ld"],
            "R": ["st", "mp", "pj", "h0", "h1", "cv", "tr", "xb", "ld"],
            "S": ["st", "mp", "pj", "h0", "h1", "tr", "cv", "xb", "ld"],
            "T": ["st", "mp", "cv", "h0", "h1", "pj", "tr", "xb", "ld"],
            "U": ["st", "mp", "h0", "h1", "pj", "cv", "tr", "xb", "ld"],
            "Z": ["ld", "st", "mp", "pj", "h0", "h1", "tr", "cv", "xb"],
            "Y": ["ld", "xb", "st", "mp", "pj", "h0", "h1", "tr", "cv"],
        }
        if kwarm:
            # keep PE busy from t=0 so the HAM clock gate releases before the
            # first conv group arrives (pg psum bank is unused during fill)
            warm_ps = pg_ps_pool.tile([112, GROUP], dt.float32, name="p_ps", tag="p")
            for _ in range(kwarm):
                nc.tensor.matmul(warm_ps[:, 0:64], lhsT=hwT_sb[:, 0:112],
                                 rhs=hwT_sb[:, 0:64], start=True, stop=True)

        if kedge == "1":
            base = [(0, 2), (2, 2)] + [(4 + 4 * i, 4) for i in range(6)] + [(28, 2), (30, 2)]
        elif kedge == "2":
            # drain-only: halve just the final group
            base = [(4 * i, 4) for i in range(NG - 1)] + [(28, 2), (30, 2)]
        else:
            base = [(NST * i, NST) for i in range(NG)]
        sched = []
        for r in range(reps):
            sched.extend(base)
        NGR = len(sched)

        def emit_fillers(n):
            # independent junk matmuls over wbig into the (yet unused) p bank:
            # keeps the PE clock hot across fill-phase gaps
            warm_ps = pg_ps_pool.tile([112, GROUP], dt.float32, name="p_ps", tag="p")
            for _ in range(n):
                nc.tensor.matmul(warm_ps[:, 0:128], lhsT=wbig_sb[0:128, 0, 0:112],
                                 rhs=wbig_sb[:, 0, 0:128], start=True, stop=True)

        if kv5:
            # v5 driver: whole-core SBUF residency for the input stream. All
            # loads + xbar transposes are issued upfront (xg/xt pools sized to
            # NGR bufs), so the steady-state loop carries only compute stages
            # and stores — no DMA latency inside the per-iteration chain.
            if kpre:
                load_early_consts()
            for g in range(NGR):
                s0_load(g)
            for g in range(NGR):
                s1_xbar(g)
            load_late_consts()
            if kfill:
                emit_fillers(kfill)
            FN5 = {"cv": s2_conv, "mp": s3_maxpool, "tr": s3b_htr,
                   "h0": lambda g: s4_highway(g, 0),
                   "h1": lambda g: s4_highway(g, 1),
                   "pj": s5_proj, "st": s6_store}
            SK5 = {}
            for ent in kv5sk.split(","):
                k, v = ent.split(":")
                SK5[k] = int(v)
            ORD5 = kv5ord.split(",")
            for it in range(NGR + max(SK5.values()) + 1):
                for key in ORD5:
                    g = it - SK5[key]
                    if 0 <= g < NGR:
                        FN5[key](g)
        else:
            if kpre:
                load_early_consts()
            if kfillpre:
                emit_fillers(kfillpre)
            for it in range(NGR + 8):
                if it == 2:
                    load_late_consts()
                for key in ORDERS[kord]:
                    skew, fn = STAGES[key]
                    g = it - skew
                    if 0 <= g < NGR:
                        fn(g)
                if kfill and it < 4:
                    emit_fillers(kfill if it < 2 else kfill // 2)

    nc.compile()
    return nc


def _prep_weights(inputs):
    W = np.zeros((FEAT, CONV_COLS), np.float32)
    cb = np.zeros(LAST_DIM, np.float32)
    off = 0
    offc = 0
    for p_pad, flist in SEGS:
        for w, c in flist:
            i = w  # filter index == width for this problem
            p_i = BYTE_LEN - w + 1
            cw = np.asarray(inputs[f"conv_w{i}"], np.float32)  # [c, EMB, w]
            for p in range(p_pad):
                sp = p if p < p_i else 0  # duplicate position 0 as padding
                for k in range(w):
                    byte = sp + k
                    W[byte * EMB:(byte + 1) * EMB,
                      off + p:off + c * p_pad:p_pad] = cw[:, :, k].T
            cb[offc:offc + c] = np.asarray(inputs[f"conv_b{i}"], np.float32)
            off += c * p_pad
            offc += c
    wbig = np.ascontiguousarray(
        W.reshape(4, 128, CONV_COLS).transpose(1, 0, 2).reshape(128, 4 * CONV_COLS)
    ).astype(bf16)
    hwT = np.concatenate([np.asarray(inputs["hw_w1"], np.float32).T,
                          np.asarray(inputs["hw_w2"], np.float32).T], 1)
    hwT = np.ascontiguousarray(hwT).astype(bf16)  # [112, 448]
    pwT = np.zeros((128, 512), np.float32)
    pwT[:112] = np.asarray(inputs["proj_w"], np.float32).T
    pwT[112] = np.asarray(inputs["proj_b"], np.float32)
    pwT = np.ascontiguousarray(pwT).astype(bf16)
    hb1 = np.asarray(inputs["hw_b1"], np.float32)
    hb2 = np.asarray(inputs["hw_b2"], np.float32)
    hbias = np.stack([hb1[:112], hb1[112:], hb2[:112], hb2[112:]], 1)  # [112, 4]
    hbias = np.ascontiguousarray(hbias)
    return wbig, hwT, pwT, cb.reshape(112, 1), hbias


def _in_maps(inputs):
    wbig, hwT, pwT, cb, hbias = _prep_weights(inputs)
    ident = np.eye(128, dtype=bf16)
    feats = np.ascontiguousarray(
        np.asarray(inputs["features"], np.float32).reshape(B * T, FEAT)
    )
    return [{
        "features": feats[c * S_PER_CORE:(c + 1) * S_PER_CORE],
        "wbig": wbig, "hwT": hwT, "pwT": pwT, "cbias": cb, "hbias": hbias,
        "ident": ident,
    } for c in range(N_CORES)]


def kernel(**inputs) -> np.ndarray:
    from concourse.bass_utils import run_bass_kernel_spmd

    if "nc" not in _cache:
        _cache["nc"] = _build()
    nc = _cache["nc"]

    in_maps = _in_maps(inputs)
    res = run_bass_kernel_spmd(nc, in_maps, core_ids=list(range(N_CORES)))
    out = np.concatenate([res.results[c]["out"] for c in range(N_CORES)], 0)
    return np.ascontiguousarray(out.reshape(B, T, OUT_DIM)).astype(np.float32)

